# revision 17
# baseline (speedup 1.0000x reference)
"""Trainium2 Bass kernel for GQA attention block (nn_Attention_81372450390110).

Module: y = AttnOut(x) with q/k RMSNorm + interleaved RoPE + causal GQA
(NH=16 q heads, KVH=4 kv heads, HD=128, D=2048, B=2, S=2048).

Sharding: 8 cores = 2 batches x 4 KV groups. Core c handles batch c//4 and
KV group c%4 (4 q heads + 1 kv head). Each core computes a full [S, D]
partial of the output projection (row-parallel over heads); the host sums
the 4 group-partials per batch.

Layout strategy (all feature-major, "transposed"):
  - host passes xT = x[b].T so the D contraction dim lands on partitions
  - qT/kT computed as [HD, S] directly (lhsT = weight chunk)
  - scores computed transposed: sT[k, q] = kT_blk.T @ qT_blk
  - softmax without max-subtraction (rmsnorm bounds |scores| <= sqrt(HD))
  - P@V untransposed (lhsT = P chunk, rhs = v block augmented with a ones
    column) -> out [q, HD | l]: the softmax denominator l falls out as the
    129th column, normalized via per-partition tensor_scalar, then the
    [128,128] tile is PE-transposed into attn_outT for the o-projection
  - cross-partition sums/broadcasts via ones-matmuls on TensorE
  - RoPE+norm-weight folded into host-precomputed coefficient tiles, with
    an even/odd deinterleaving permutation baked into wq/wk columns
"""

import os
import sys

sys.path.insert(0, "/opt/trn_rl_repo")

import numpy as np
import ml_dtypes

BF16 = ml_dtypes.bfloat16

B = 2
S = 2048
D = 2048
NH = 16
KVH = 4
HD = 128
THETA = 10000.0
EPS = 1e-6
NHL = NH // KVH  # q heads per core (4)
SCALE = 1.0 / float(np.sqrt(HD))

_CACHED = {}


def build_nc(s=S, d=D, nhl=NHL, hd=HD):
    import concourse.mybir as mybir
    import concourse.tile as tile
    from concourse import bacc

    f32 = mybir.dt.float32
    bf16 = mybir.dt.bfloat16
    AF = mybir.ActivationFunctionType

    kc_n = d // 128          # contraction chunks for projections
    nb_n = s // 512          # 512-token blocks
    qt_n = s // 512          # q tiles (512 wide) in attention
    kb_n = s // 128          # k blocks (128 wide)

    nc = bacc.Bacc("TRN2", target_bir_lowering=False, debug=False)

    xT_d = nc.dram_tensor("xT", (d, s), bf16, kind="ExternalInput")
    wq_d = nc.dram_tensor("wq", (d, nhl * hd), bf16, kind="ExternalInput")
    wk_d = nc.dram_tensor("wk", (d, hd), bf16, kind="ExternalInput")
    wv_d = nc.dram_tensor("wv", (d, hd), bf16, kind="ExternalInput")
    wo_d = nc.dram_tensor("wo", (nhl * hd, d), bf16, kind="ExternalInput")
    m1q_d = nc.dram_tensor("m1q", (hd, s), f32, kind="ExternalInput")
    m2q_d = nc.dram_tensor("m2q", (hd, s), f32, kind="ExternalInput")
    m1k_d = nc.dram_tensor("m1k", (hd, s), f32, kind="ExternalInput")
    m2k_d = nc.dram_tensor("m2k", (hd, s), f32, kind="ExternalInput")
    tri_d = nc.dram_tensor("tri", (128, 128), bf16, kind="ExternalInput")
    y_d = nc.dram_tensor("y", (s, d), f32, kind="ExternalOutput")

    with tile.TileContext(nc) as tc, nc.allow_low_precision(
        reason="bf16 compute by design; fp32 accumulation in PSUM"
    ):
        with (
            tc.tile_pool(name="const", bufs=1) as const,
            tc.tile_pool(name="persist", bufs=1) as persist,
        ):
            # ---- resident weights / coefficients -------------------------
            wq_sb = persist.tile([128, kc_n, nhl * hd], bf16, tag="wq")
            wq_re = wq_d.rearrange("(kc p) m -> p kc m", p=128)
            for kc in range(kc_n):
                nc.sync.dma_start(wq_sb[:, kc, :], wq_re[:, kc, :])
            wk_sb = persist.tile([128, kc_n, hd], bf16, tag="wk")
            wk_re = wk_d.rearrange("(kc p) m -> p kc m", p=128)
            wv_sb = persist.tile([128, kc_n, hd], bf16, tag="wv")
            wv_re = wv_d.rearrange("(kc p) m -> p kc m", p=128)
            for kc in range(kc_n):
                nc.sync.dma_start(wk_sb[:, kc, :], wk_re[:, kc, :])
                nc.sync.dma_start(wv_sb[:, kc, :], wv_re[:, kc, :])
            wo_sb = persist.tile([128, nhl, d], bf16, tag="wo")
            wo_re = wo_d.rearrange("(h p) m -> p h m", p=128)
            for h in range(nhl):
                nc.sync.dma_start(wo_sb[:, h, :], wo_re[:, h, :])

            m1q_sb = persist.tile([hd, s], f32, tag="m1q")
            m2q_sb = persist.tile([hd, s], f32, tag="m2q")
            m1k_sb = persist.tile([hd, s], f32, tag="m1k")
            m2k_sb = persist.tile([hd, s], f32, tag="m2k")
            nc.sync.dma_start(m1q_sb[:], m1q_d[:])
            nc.sync.dma_start(m2q_sb[:], m2q_d[:])
            nc.sync.dma_start(m1k_sb[:], m1k_d[:])
            nc.sync.dma_start(m2k_sb[:], m2k_d[:])
            tri_sb = const.tile([128, 128], bf16, tag="tri")
            nc.sync.dma_start(tri_sb[:], tri_d[:])

            ones_k = const.tile([128, 1], bf16, tag="ones_k")
            nc.vector.memset(ones_k[:], 1.0)
            ones_1 = const.tile([1, 128], bf16, tag="ones_1")
            nc.vector.memset(ones_1[:], 1.0)
            eps_sb = const.tile([128, 1], f32, tag="eps")
            nc.vector.memset(eps_sb[:], EPS)
            ident = const.tile([128, 128], bf16, tag="ident")
            from concourse.masks import make_identity
            make_identity(nc, ident[:])

            # ---- persistent activations ---------------------------------
            qT_sb = [persist.tile([hd, s], bf16, tag=f"qT{h}", name=f"qT{h}") for h in range(nhl)]
            kT_sb = persist.tile([hd, s], bf16, tag="kT")
            v_sb = persist.tile([128, kb_n, hd + 1], bf16, tag="v")
            nc.vector.memset(v_sb[:, :, hd:hd + 1], 1.0)
            attT_sb = [persist.tile([hd, s], bf16, tag=f"attT{h}", name=f"attT{h}") for h in range(nhl)]

            # ================= Phase A: projections + norm + rope =========
            with (
                tc.tile_pool(name="xtp", bufs=2) as xtp,
                tc.tile_pool(name="workA", bufs=3) as wa,
                tc.tile_pool(name="psA", bufs=2, space="PSUM") as psA,
            ):
                xT_re = xT_d.rearrange("(kc p) n -> p kc n", p=128)

                def norm_rope_chain(q_ps, t, cs):
                    # rmsnorm via ones-matmul + bcast-matmul, rope via coeff
                    # tiles; PE ops here are emitted one tensor late so the
                    # ACT/DVE chain overlaps the next tensor's projection MMs.
                    sq = wa.tile([128, 512], bf16, tag="sq", name="sq")
                    nc.scalar.activation(sq[:], q_ps[:], AF.Square)
                    ssq = psA.tile([1, 512], f32, tag="ssq", name="ssq", bufs=1)
                    nc.tensor.matmul(ssq[:], ones_k[:], sq[:])
                    ssq_bf = wa.tile([1, 512], bf16, tag="ssq_bf", name="ssq_bf")
                    nc.scalar.copy(ssq_bf[:], ssq[:])
                    rb_ps = psA.tile([128, 512], f32, tag="rb_ps", name="rb_ps", bufs=1)
                    nc.tensor.matmul(rb_ps[:], ones_1[:], ssq_bf[:])
                    tmp = wa.tile([128, 512], f32, tag="tmp", name="tmp")
                    nc.scalar.activation(
                        tmp[:], rb_ps[:], AF.Sqrt, scale=1.0 / hd, bias=eps_sb[:]
                    )
                    rb = wa.tile([128, 512], f32, tag="rb", name="rb")
                    nc.vector.reciprocal_approx_fast(rb[:], tmp[:])
                    qn = wa.tile([128, 512], f32, tag="qn", name="qn")
                    nc.vector.tensor_mul(qn[:], q_ps[:], rb[:])
                    qs = wa.tile([128, 512], f32, tag="qs", name="qs")
                    nc.sync.dma_start(qs[0:64, :], qn[64:128, :])
                    nc.sync.dma_start(qs[64:128, :], qn[0:64, :])
                    m1 = m1q_sb if t < nhl else m1k_sb
                    m2 = m2q_sb if t < nhl else m2k_sb
                    t1 = wa.tile([128, 512], f32, tag="t1", name="t1")
                    nc.vector.tensor_mul(t1[:], qn[:], m1[:, cs])
                    t2 = wa.tile([128, 512], f32, tag="t2", name="t2")
                    nc.vector.tensor_mul(t2[:], qs[:], m2[:, cs])
                    dest = qT_sb[t] if t < nhl else kT_sb
                    nc.vector.tensor_add(dest[:, cs], t1[:], t2[:])

                pending = None
                for nb in range(nb_n):
                    cs = slice(nb * 512, (nb + 1) * 512)
                    xt = xtp.tile([128, kc_n, 512], bf16, tag="xt")
                    for kc in range(kc_n):
                        nc.sync.dma_start(xt[:, kc, :], xT_re[:, kc, cs])

                    # k first (unblocks attention earliest), then q heads
                    for t in [nhl] + list(range(nhl)):
                        q_ps = psA.tile([128, 512], f32, tag="q_ps", bufs=3)
                        for kc in range(kc_n):
                            if t < nhl:
                                lhsT = wq_sb[:, kc, t * hd:(t + 1) * hd]
                            else:
                                lhsT = wk_sb[:, kc, :]
                            nc.tensor.matmul(
                                q_ps[:], lhsT, xt[:, kc, :],
                                start=(kc == 0), stop=(kc == kc_n - 1),
                            )
                        if pending is not None:
                            norm_rope_chain(*pending)
                        pending = (q_ps, t, cs)

                    # v: plain projection, token-major
                    for tt in range(4):
                        v_ps = psA.tile([128, hd], f32, tag="v_ps", bufs=2)
                        for kc in range(kc_n):
                            nc.tensor.matmul(
                                v_ps[:],
                                xt[:, kc, tt * 128:(tt + 1) * 128],
                                wv_sb[:, kc, :],
                                start=(kc == 0), stop=(kc == kc_n - 1),
                            )
                        nc.vector.tensor_copy(v_sb[:, nb * 4 + tt, 0:hd], v_ps[:])
                if pending is not None:
                    norm_rope_chain(*pending)

            # ================= Phase B: causal flash attention ============
            with (
                tc.tile_pool(name="workB", bufs=3) as wb,
                tc.tile_pool(name="psB", bufs=2, space="PSUM") as psB,
            ):
                for h in range(nhl):
                    for qt in range(qt_n):
                        qcs = slice(qt * 512, (qt + 1) * 512)
                        nkb = 4 * qt + 4
                        att_ps = [
                            psB.tile([128, hd + 1], f32, tag="att", bufs=4,
                                     name=f"att{qs}")
                            for qs in range(4)
                        ]
                        s_tiles = {}

                        def emit_s(kb):
                            sp = psB.tile([128, 512], f32, tag="s_ps", name="s_ps", bufs=4)
                            r = kb - 4 * qt
                            c0 = 128 * r if r > 0 else 0
                            nc.tensor.matmul(
                                sp[:, c0:512],
                                kT_sb[:, kb * 128:(kb + 1) * 128],
                                qT_sb[h][:, qt * 512 + c0:(qt + 1) * 512],
                            )
                            s_tiles[kb] = sp

                        emit_s(0)
                        if nkb > 1:
                            emit_s(1)
                        for kb in range(nkb):
                            if kb + 2 < nkb:
                                emit_s(kb + 2)
                            sp = s_tiles.pop(kb)
                            p = wb.tile([128, 512], bf16, tag="p")
                            r = kb - 4 * qt
                            if r >= 0:
                                nc.scalar.activation(
                                    p[:, 128 * r:512], sp[:, 128 * r:512],
                                    AF.Exp, scale=SCALE,
                                )
                                nc.vector.tensor_mul(
                                    p[:, 128 * r:128 * (r + 1)],
                                    p[:, 128 * r:128 * (r + 1)],
                                    tri_sb[:],
                                )
                            else:
                                nc.scalar.activation(p[:], sp[:], AF.Exp, scale=SCALE)
                            for qs in range(4):
                                kmax = 4 * qt + qs
                                if kb > kmax:
                                    continue
                                nc.tensor.matmul(
                                    att_ps[qs][:],
                                    p[:, qs * 128:(qs + 1) * 128],
                                    v_sb[:, kb, :],
                                    start=(kb == 0), stop=(kb == kmax),
                                )
                        for qs in range(4):
                            rec = wb.tile([128, 1], f32, tag="rec")
                            nc.vector.reciprocal(rec[:], att_ps[qs][:, hd:hd + 1])
                            att_n = wb.tile([128, 128], bf16, tag="att_n")
                            nc.vector.tensor_scalar_mul(
                                att_n[:], att_ps[qs][:, 0:hd], rec[:]
                            )
                            nc.sync.dma_start_transpose(
                                attT_sb[h][:, qt * 512 + qs * 128:
                                           qt * 512 + (qs + 1) * 128],
                                att_n[:],
                            )

            # ================= Phase C: output projection =================
            with (
                tc.tile_pool(name="workC", bufs=3) as wc,
                tc.tile_pool(name="psC", bufs=2, space="PSUM") as psC,
            ):
                for tt in range(s // 128):
                    for db in range(d // 512):
                        y_ps = psC.tile([128, 512], f32, tag="y")
                        for h in range(nhl):
                            nc.tensor.matmul(
                                y_ps[:],
                                attT_sb[h][:, tt * 128:(tt + 1) * 128],
                                wo_sb[:, h, db * 512:(db + 1) * 512],
                                start=(h == 0), stop=(h == nhl - 1),
                            )
                        y_sb = wc.tile([128, 512], f32, tag="ysb", name="ysb")
                        if db % 2 == 0:
                            nc.scalar.copy(y_sb[:], y_ps[:])
                        else:
                            nc.vector.tensor_copy(y_sb[:], y_ps[:])
                        nc.sync.dma_start(
                            y_d[tt * 128:(tt + 1) * 128, db * 512:(db + 1) * 512],
                            y_sb[:],
                        )

    nc.compile()
    return nc


def _rope_coeffs(norm_w, s=S, hd=HD):
    """Coefficient tiles [hd, s] folding rope cos/sin + permuted norm weight."""
    perm = np.concatenate([np.arange(0, hd, 2), np.arange(1, hd, 2)])
    w = np.asarray(norm_w, np.float64)[perm]
    half = hd // 2
    pos = np.arange(s, dtype=np.float64)
    inv_freq = 1.0 / (THETA ** (np.arange(0, hd, 2, dtype=np.float64) / hd))
    ang = pos[None, :] * inv_freq[:, None]          # [half, s]
    cos, sin = np.cos(ang), np.sin(ang)
    m1 = np.empty((hd, s), np.float32)
    m2 = np.empty((hd, s), np.float32)
    m1[:half] = cos * w[:half, None]
    m1[half:] = cos * w[half:, None]
    m2[:half] = -sin * w[half:, None]
    m2[half:] = sin * w[:half, None]
    return m1, m2


def _host_prep(x, wq, wk, wv, wo, q_norm_w, k_norm_w):
    perm = np.concatenate([np.arange(0, HD, 2), np.arange(1, HD, 2)])
    m1q, m2q = _rope_coeffs(q_norm_w)
    m1k, m2k = _rope_coeffs(k_norm_w)
    tri = np.triu(np.ones((128, 128), np.float32)).astype(BF16)

    in_maps = []
    for c in range(8):
        b, g = c // 4, c % 4
        heads = range(NHL * g, NHL * g + NHL)
        wq_loc = np.concatenate(
            [wq[:, h * HD:(h + 1) * HD][:, perm] for h in heads], axis=1
        )
        in_maps.append({
            "xT": np.ascontiguousarray(x[b].T).astype(BF16),
            "wq": np.ascontiguousarray(wq_loc).astype(BF16),
            "wk": np.ascontiguousarray(wk[:, g * HD:(g + 1) * HD][:, perm]).astype(BF16),
            "wv": np.ascontiguousarray(wv[:, g * HD:(g + 1) * HD]).astype(BF16),
            "wo": np.ascontiguousarray(wo[NHL * g * HD:NHL * (g + 1) * HD, :]).astype(BF16),
            "m1q": m1q, "m2q": m2q, "m1k": m1k, "m2k": m2k,
            "tri": tri,
        })
    return in_maps


def _install_ntff_shim():
    import types
    if "antenv.axon_hooks" in sys.modules:
        return
    mod = types.ModuleType("antenv.axon_hooks")
    _hook = [None]
    mod.set_axon_ntff_profile_hook = lambda h: _hook.__setitem__(0, h)
    mod.get_axon_ntff_profile_hook = lambda: _hook[0]
    sys.modules["antenv.axon_hooks"] = mod
    try:
        from trn_agent_boot.trn_boot import _ntff_profile_via_ctypes
        mod.set_axon_ntff_profile_hook(
            _ntff_profile_via_ctypes("/opt/axon/libaxon_pjrt.so")
        )
    except Exception:
        pass


LAST_EXEC_NS = None


def kernel(x, wq, wk, wv, wo, q_norm_w, k_norm_w):
    global LAST_EXEC_NS
    from concourse import bass_utils

    x = np.asarray(x)
    if "nc" not in _CACHED:
        _CACHED["nc"] = build_nc()
    nc = _CACHED["nc"]

    in_maps = _host_prep(
        np.asarray(x, np.float32), np.asarray(wq, np.float32),
        np.asarray(wk, np.float32), np.asarray(wv, np.float32),
        np.asarray(wo, np.float32), np.asarray(q_norm_w, np.float32),
        np.asarray(k_norm_w, np.float32),
    )
    trace = bool(int(os.environ.get("BASS_KERNEL_TRACE", "0")))
    if trace:
        _install_ntff_shim()
    res = bass_utils.run_bass_kernel_spmd(
        nc, in_maps, core_ids=list(range(8)), trace=trace
    )
    LAST_EXEC_NS = res.exec_time_ns
    y = np.zeros((B, S, D), np.float32)
    for c in range(8):
        y[c // 4] += res.results[c]["y"]
    return y


# revision 18
# speedup vs baseline: 1.0389x; 1.0389x over previous
"""Trainium2 Bass kernel for GQA attention block (nn_Attention_81372450390110).

Module: y = AttnOut(x) with q/k RMSNorm + interleaved RoPE + causal GQA
(NH=16 q heads, KVH=4 kv heads, HD=128, D=2048, B=2, S=2048).

Sharding: 8 cores = 2 batches x 4 KV groups. Core c handles batch c//4 and
KV group c%4 (4 q heads + 1 kv head). Each core computes a full [S, D]
partial of the output projection (row-parallel over heads); the host sums
the 4 group-partials per batch.

Layout strategy (all feature-major, "transposed"):
  - host passes xT = x[b].T so the D contraction dim lands on partitions
  - qT/kT computed as [HD, S] directly (lhsT = weight chunk)
  - scores computed transposed: sT[k, q] = kT_blk.T @ qT_blk
  - softmax without max-subtraction (rmsnorm bounds |scores| <= sqrt(HD))
  - P@V untransposed (lhsT = P chunk, rhs = v block augmented with a ones
    column) -> out [q, HD | l]: the softmax denominator l falls out as the
    129th column, normalized via per-partition tensor_scalar, then the
    [128,128] tile is PE-transposed into attn_outT for the o-projection
  - cross-partition sums/broadcasts via ones-matmuls on TensorE
  - RoPE+norm-weight folded into host-precomputed coefficient tiles, with
    an even/odd deinterleaving permutation baked into wq/wk columns
"""

import os
import sys

sys.path.insert(0, "/opt/trn_rl_repo")

import numpy as np
import ml_dtypes

BF16 = ml_dtypes.bfloat16

B = 2
S = 2048
D = 2048
NH = 16
KVH = 4
HD = 128
THETA = 10000.0
EPS = 1e-6
NHL = NH // KVH  # q heads per core (4)
SCALE = 1.0 / float(np.sqrt(HD))

_CACHED = {}


def build_nc(s=S, d=D, nhl=NHL, hd=HD):
    import concourse.mybir as mybir
    import concourse.tile as tile
    from concourse import bacc

    f32 = mybir.dt.float32
    bf16 = mybir.dt.bfloat16
    AF = mybir.ActivationFunctionType

    kc_n = d // 128          # contraction chunks for projections
    nb_n = s // 512          # 512-token blocks
    qt_n = s // 512          # q tiles (512 wide) in attention
    kb_n = s // 128          # k blocks (128 wide)

    nc = bacc.Bacc("TRN2", target_bir_lowering=False, debug=False)

    xT_d = nc.dram_tensor("xT", (d, s), bf16, kind="ExternalInput")
    wq_d = nc.dram_tensor("wq", (d, nhl * hd), bf16, kind="ExternalInput")
    wk_d = nc.dram_tensor("wk", (d, hd), bf16, kind="ExternalInput")
    wv_d = nc.dram_tensor("wv", (d, hd), bf16, kind="ExternalInput")
    wo_d = nc.dram_tensor("wo", (nhl * hd, d), bf16, kind="ExternalInput")
    m1q_d = nc.dram_tensor("m1q", (hd, s), f32, kind="ExternalInput")
    m2q_d = nc.dram_tensor("m2q", (hd, s), f32, kind="ExternalInput")
    m1k_d = nc.dram_tensor("m1k", (hd, s), f32, kind="ExternalInput")
    m2k_d = nc.dram_tensor("m2k", (hd, s), f32, kind="ExternalInput")
    tri_d = nc.dram_tensor("tri", (128, 128), bf16, kind="ExternalInput")
    y_d = nc.dram_tensor("y", (s, d), f32, kind="ExternalOutput")

    with tile.TileContext(nc) as tc, nc.allow_low_precision(
        reason="bf16 compute by design; fp32 accumulation in PSUM"
    ):
        with (
            tc.tile_pool(name="const", bufs=1) as const,
            tc.tile_pool(name="persist", bufs=1) as persist,
        ):
            # ---- resident weights / coefficients -------------------------
            wq_sb = persist.tile([128, kc_n, nhl * hd], bf16, tag="wq")
            wq_re = wq_d.rearrange("(kc p) m -> p kc m", p=128)
            for kc in range(kc_n):
                nc.sync.dma_start(wq_sb[:, kc, :], wq_re[:, kc, :])
            wk_sb = persist.tile([128, kc_n, hd], bf16, tag="wk")
            wk_re = wk_d.rearrange("(kc p) m -> p kc m", p=128)
            wv_sb = persist.tile([128, kc_n, hd], bf16, tag="wv")
            wv_re = wv_d.rearrange("(kc p) m -> p kc m", p=128)
            for kc in range(kc_n):
                nc.sync.dma_start(wk_sb[:, kc, :], wk_re[:, kc, :])
                nc.sync.dma_start(wv_sb[:, kc, :], wv_re[:, kc, :])
            wo_sb = persist.tile([128, nhl, d], bf16, tag="wo")
            wo_re = wo_d.rearrange("(h p) m -> p h m", p=128)
            for h in range(nhl):
                nc.sync.dma_start(wo_sb[:, h, :], wo_re[:, h, :])

            m1q_sb = persist.tile([hd, s], f32, tag="m1q")
            m2q_sb = persist.tile([hd, s], f32, tag="m2q")
            m1k_sb = persist.tile([hd, s], f32, tag="m1k")
            m2k_sb = persist.tile([hd, s], f32, tag="m2k")
            nc.sync.dma_start(m1q_sb[:], m1q_d[:])
            nc.sync.dma_start(m2q_sb[:], m2q_d[:])
            nc.sync.dma_start(m1k_sb[:], m1k_d[:])
            nc.sync.dma_start(m2k_sb[:], m2k_d[:])
            tri_sb = const.tile([128, 128], bf16, tag="tri")
            nc.sync.dma_start(tri_sb[:], tri_d[:])

            ones_k = const.tile([128, 1], bf16, tag="ones_k")
            nc.vector.memset(ones_k[:], 1.0)
            ones_1 = const.tile([1, 128], bf16, tag="ones_1")
            nc.vector.memset(ones_1[:], 1.0)
            eps_sb = const.tile([128, 1], f32, tag="eps")
            nc.vector.memset(eps_sb[:], EPS)
            ident = const.tile([128, 128], bf16, tag="ident")
            from concourse.masks import make_identity
            make_identity(nc, ident[:])

            # ---- persistent activations ---------------------------------
            qT_sb = [persist.tile([hd, s], bf16, tag=f"qT{h}", name=f"qT{h}") for h in range(nhl)]
            kT_sb = persist.tile([hd, s], bf16, tag="kT")
            v_sb = persist.tile([128, kb_n, hd + 1], bf16, tag="v")
            nc.vector.memset(v_sb[:, :, hd:hd + 1], 1.0)
            attT_sb = [persist.tile([hd, s], bf16, tag=f"attT{h}", name=f"attT{h}") for h in range(nhl)]

            # ================= Phase A: projections + norm + rope =========
            with (
                tc.tile_pool(name="xtp", bufs=2) as xtp,
                tc.tile_pool(name="workA", bufs=3) as wa,
                tc.tile_pool(name="psA", bufs=2, space="PSUM") as psA,
            ):
                xT_re = xT_d.rearrange("(kc p) n -> p kc n", p=128)

                def norm_rope_chain(q_ps, t, cs):
                    # rmsnorm via ones-matmul + bcast-matmul, rope via coeff
                    # tiles; PE ops here are emitted one tensor late so the
                    # ACT/DVE chain overlaps the next tensor's projection MMs.
                    sq = wa.tile([128, 512], bf16, tag="sq", name="sq")
                    nc.scalar.activation(sq[:], q_ps[:], AF.Square)
                    ssq = psA.tile([1, 512], f32, tag="ssq", name="ssq", bufs=1)
                    nc.tensor.matmul(ssq[:], ones_k[:], sq[:])
                    ssq_bf = wa.tile([1, 512], bf16, tag="ssq_bf", name="ssq_bf")
                    nc.scalar.copy(ssq_bf[:], ssq[:])
                    rb_ps = psA.tile([128, 512], f32, tag="rb_ps", name="rb_ps", bufs=1)
                    nc.tensor.matmul(rb_ps[:], ones_1[:], ssq_bf[:])
                    tmp = wa.tile([128, 512], f32, tag="tmp", name="tmp")
                    nc.scalar.activation(
                        tmp[:], rb_ps[:], AF.Sqrt, scale=1.0 / hd, bias=eps_sb[:]
                    )
                    rb = wa.tile([128, 512], f32, tag="rb", name="rb")
                    nc.vector.reciprocal_approx_fast(rb[:], tmp[:])
                    qn = wa.tile([128, 512], f32, tag="qn", name="qn")
                    nc.vector.tensor_mul(qn[:], q_ps[:], rb[:])
                    qs = wa.tile([128, 512], f32, tag="qs", name="qs")
                    nc.sync.dma_start(qs[0:64, :], qn[64:128, :])
                    nc.sync.dma_start(qs[64:128, :], qn[0:64, :])
                    m1 = m1q_sb if t < nhl else m1k_sb
                    m2 = m2q_sb if t < nhl else m2k_sb
                    t1 = wa.tile([128, 512], f32, tag="t1", name="t1")
                    nc.vector.tensor_mul(t1[:], qn[:], m1[:, cs])
                    t2 = wa.tile([128, 512], f32, tag="t2", name="t2")
                    nc.vector.tensor_mul(t2[:], qs[:], m2[:, cs])
                    dest = qT_sb[t] if t < nhl else kT_sb
                    nc.vector.tensor_add(dest[:, cs], t1[:], t2[:])

                pending = None
                for nb in range(nb_n):
                    cs = slice(nb * 512, (nb + 1) * 512)
                    xt = xtp.tile([128, kc_n, 512], bf16, tag="xt")
                    for kc in range(kc_n):
                        nc.sync.dma_start(xt[:, kc, :], xT_re[:, kc, cs])

                    # k first (unblocks attention earliest), then q heads
                    for t in [nhl] + list(range(nhl)):
                        q_ps = psA.tile([128, 512], f32, tag="q_ps", bufs=3)
                        for kc in range(kc_n):
                            if t < nhl:
                                lhsT = wq_sb[:, kc, t * hd:(t + 1) * hd]
                            else:
                                lhsT = wk_sb[:, kc, :]
                            nc.tensor.matmul(
                                q_ps[:], lhsT, xt[:, kc, :],
                                start=(kc == 0), stop=(kc == kc_n - 1),
                            )
                        if pending is not None:
                            norm_rope_chain(*pending)
                        pending = (q_ps, t, cs)

                    # v: plain projection, token-major
                    for tt in range(4):
                        v_ps = psA.tile([128, hd], f32, tag="v_ps", bufs=2)
                        for kc in range(kc_n):
                            nc.tensor.matmul(
                                v_ps[:],
                                xt[:, kc, tt * 128:(tt + 1) * 128],
                                wv_sb[:, kc, :],
                                start=(kc == 0), stop=(kc == kc_n - 1),
                            )
                        nc.vector.tensor_copy(v_sb[:, nb * 4 + tt, 0:hd], v_ps[:])
                if pending is not None:
                    norm_rope_chain(*pending)

            # ================= Phase B: causal flash attention ============
            with (
                tc.tile_pool(name="workB", bufs=3) as wb,
                tc.tile_pool(name="psB", bufs=2, space="PSUM") as psB,
            ):
                for h in range(nhl):
                    for qt in range(qt_n):
                        qcs = slice(qt * 512, (qt + 1) * 512)
                        nkb = 4 * qt + 4
                        att_ps = [
                            psB.tile([128, hd + 1], f32, tag="att", bufs=4,
                                     name=f"att{qs}")
                            for qs in range(4)
                        ]
                        s_tiles = {}

                        def emit_s(kb):
                            sp = psB.tile([128, 512], f32, tag="s_ps", name="s_ps", bufs=3)
                            r = kb - 4 * qt
                            c0 = 128 * r if r > 0 else 0
                            nc.tensor.matmul(
                                sp[:, c0:512],
                                kT_sb[:, kb * 128:(kb + 1) * 128],
                                qT_sb[h][:, qt * 512 + c0:(qt + 1) * 512],
                            )
                            s_tiles[kb] = sp

                        emit_s(0)
                        if nkb > 1:
                            emit_s(1)
                        for kb in range(nkb):
                            if kb + 2 < nkb:
                                emit_s(kb + 2)
                            sp = s_tiles.pop(kb)
                            p = wb.tile([128, 512], bf16, tag="p")
                            r = kb - 4 * qt
                            if r >= 0:
                                nc.scalar.activation(
                                    p[:, 128 * r:512], sp[:, 128 * r:512],
                                    AF.Exp, scale=SCALE,
                                )
                                nc.vector.tensor_mul(
                                    p[:, 128 * r:128 * (r + 1)],
                                    p[:, 128 * r:128 * (r + 1)],
                                    tri_sb[:],
                                )
                            else:
                                nc.scalar.activation(p[:], sp[:], AF.Exp, scale=SCALE)
                            for qs in range(4):
                                kmax = 4 * qt + qs
                                if kb > kmax:
                                    continue
                                nc.tensor.matmul(
                                    att_ps[qs][:],
                                    p[:, qs * 128:(qs + 1) * 128],
                                    v_sb[:, kb, :],
                                    start=(kb == 0), stop=(kb == kmax),
                                )
                        for qs in range(4):
                            rec = wb.tile([128, 1], f32, tag="rec")
                            nc.vector.reciprocal(rec[:], att_ps[qs][:, hd:hd + 1])
                            att_n = wb.tile([128, 128], bf16, tag="att_n")
                            nc.vector.tensor_scalar_mul(
                                att_n[:], att_ps[qs][:, 0:hd], rec[:]
                            )
                            tp_ps = psB.tile([128, 128], bf16, tag="tp", bufs=1)
                            nc.tensor.transpose(tp_ps[:], att_n[:], ident[:])
                            nc.vector.tensor_copy(
                                attT_sb[h][:, qt * 512 + qs * 128:
                                           qt * 512 + (qs + 1) * 128],
                                tp_ps[:],
                            )

            # ================= Phase C: output projection =================
            with (
                tc.tile_pool(name="workC", bufs=3) as wc,
                tc.tile_pool(name="psC", bufs=2, space="PSUM") as psC,
            ):
                for tt in range(s // 128):
                    for db in range(d // 512):
                        y_ps = psC.tile([128, 512], f32, tag="y")
                        for h in range(nhl):
                            nc.tensor.matmul(
                                y_ps[:],
                                attT_sb[h][:, tt * 128:(tt + 1) * 128],
                                wo_sb[:, h, db * 512:(db + 1) * 512],
                                start=(h == 0), stop=(h == nhl - 1),
                            )
                        y_sb = wc.tile([128, 512], f32, tag="ysb", name="ysb")
                        if db % 2 == 0:
                            nc.scalar.copy(y_sb[:], y_ps[:])
                        else:
                            nc.vector.tensor_copy(y_sb[:], y_ps[:])
                        nc.sync.dma_start(
                            y_d[tt * 128:(tt + 1) * 128, db * 512:(db + 1) * 512],
                            y_sb[:],
                        )

    nc.compile()
    return nc


def _rope_coeffs(norm_w, s=S, hd=HD):
    """Coefficient tiles [hd, s] folding rope cos/sin + permuted norm weight."""
    perm = np.concatenate([np.arange(0, hd, 2), np.arange(1, hd, 2)])
    w = np.asarray(norm_w, np.float64)[perm]
    half = hd // 2
    pos = np.arange(s, dtype=np.float64)
    inv_freq = 1.0 / (THETA ** (np.arange(0, hd, 2, dtype=np.float64) / hd))
    ang = pos[None, :] * inv_freq[:, None]          # [half, s]
    cos, sin = np.cos(ang), np.sin(ang)
    m1 = np.empty((hd, s), np.float32)
    m2 = np.empty((hd, s), np.float32)
    m1[:half] = cos * w[:half, None]
    m1[half:] = cos * w[half:, None]
    m2[:half] = -sin * w[half:, None]
    m2[half:] = sin * w[:half, None]
    return m1, m2


def _host_prep(x, wq, wk, wv, wo, q_norm_w, k_norm_w):
    perm = np.concatenate([np.arange(0, HD, 2), np.arange(1, HD, 2)])
    m1q, m2q = _rope_coeffs(q_norm_w)
    m1k, m2k = _rope_coeffs(k_norm_w)
    tri = np.triu(np.ones((128, 128), np.float32)).astype(BF16)

    in_maps = []
    for c in range(8):
        b, g = c // 4, c % 4
        heads = range(NHL * g, NHL * g + NHL)
        wq_loc = np.concatenate(
            [wq[:, h * HD:(h + 1) * HD][:, perm] for h in heads], axis=1
        )
        in_maps.append({
            "xT": np.ascontiguousarray(x[b].T).astype(BF16),
            "wq": np.ascontiguousarray(wq_loc).astype(BF16),
            "wk": np.ascontiguousarray(wk[:, g * HD:(g + 1) * HD][:, perm]).astype(BF16),
            "wv": np.ascontiguousarray(wv[:, g * HD:(g + 1) * HD]).astype(BF16),
            "wo": np.ascontiguousarray(wo[NHL * g * HD:NHL * (g + 1) * HD, :]).astype(BF16),
            "m1q": m1q, "m2q": m2q, "m1k": m1k, "m2k": m2k,
            "tri": tri,
        })
    return in_maps


def _install_ntff_shim():
    import types
    if "antenv.axon_hooks" in sys.modules:
        return
    mod = types.ModuleType("antenv.axon_hooks")
    _hook = [None]
    mod.set_axon_ntff_profile_hook = lambda h: _hook.__setitem__(0, h)
    mod.get_axon_ntff_profile_hook = lambda: _hook[0]
    sys.modules["antenv.axon_hooks"] = mod
    try:
        from trn_agent_boot.trn_boot import _ntff_profile_via_ctypes
        mod.set_axon_ntff_profile_hook(
            _ntff_profile_via_ctypes("/opt/axon/libaxon_pjrt.so")
        )
    except Exception:
        pass


LAST_EXEC_NS = None


def kernel(x, wq, wk, wv, wo, q_norm_w, k_norm_w):
    global LAST_EXEC_NS
    from concourse import bass_utils

    x = np.asarray(x)
    if "nc" not in _CACHED:
        _CACHED["nc"] = build_nc()
    nc = _CACHED["nc"]

    in_maps = _host_prep(
        np.asarray(x, np.float32), np.asarray(wq, np.float32),
        np.asarray(wk, np.float32), np.asarray(wv, np.float32),
        np.asarray(wo, np.float32), np.asarray(q_norm_w, np.float32),
        np.asarray(k_norm_w, np.float32),
    )
    trace = bool(int(os.environ.get("BASS_KERNEL_TRACE", "0")))
    if trace:
        _install_ntff_shim()
    res = bass_utils.run_bass_kernel_spmd(
        nc, in_maps, core_ids=list(range(8)), trace=trace
    )
    LAST_EXEC_NS = res.exec_time_ns
    y = np.zeros((B, S, D), np.float32)
    for c in range(8):
        y[c // 4] += res.results[c]["y"]
    return y


# revision 19
# speedup vs baseline: 1.0433x; 1.0043x over previous
"""Trainium2 Bass kernel for GQA attention block (nn_Attention_81372450390110).

Module: y = AttnOut(x) with q/k RMSNorm + interleaved RoPE + causal GQA
(NH=16 q heads, KVH=4 kv heads, HD=128, D=2048, B=2, S=2048).

Sharding: 8 cores = 2 batches x 4 KV groups. Core c handles batch c//4 and
KV group c%4 (4 q heads + 1 kv head). Each core computes a full [S, D]
partial of the output projection (row-parallel over heads); the host sums
the 4 group-partials per batch.

Layout strategy (all feature-major, "transposed"):
  - host passes xT = x[b].T so the D contraction dim lands on partitions
  - qT/kT computed as [HD, S] directly (lhsT = weight chunk)
  - scores computed transposed: sT[k, q] = kT_blk.T @ qT_blk
  - softmax without max-subtraction (rmsnorm bounds |scores| <= sqrt(HD))
  - P@V untransposed (lhsT = P chunk, rhs = v block augmented with a ones
    column) -> out [q, HD | l]: the softmax denominator l falls out as the
    129th column, normalized via per-partition tensor_scalar, then the
    [128,128] tile is PE-transposed into attn_outT for the o-projection
  - cross-partition sums/broadcasts via ones-matmuls on TensorE
  - RoPE+norm-weight folded into host-precomputed coefficient tiles, with
    an even/odd deinterleaving permutation baked into wq/wk columns
"""

import os
import sys

sys.path.insert(0, "/opt/trn_rl_repo")

import numpy as np
import ml_dtypes

BF16 = ml_dtypes.bfloat16

B = 2
S = 2048
D = 2048
NH = 16
KVH = 4
HD = 128
THETA = 10000.0
EPS = 1e-6
NHL = NH // KVH  # q heads per core (4)
SCALE = 1.0 / float(np.sqrt(HD))

_CACHED = {}


def build_nc(s=S, d=D, nhl=NHL, hd=HD):
    import concourse.mybir as mybir
    import concourse.tile as tile
    from concourse import bacc

    f32 = mybir.dt.float32
    bf16 = mybir.dt.bfloat16
    AF = mybir.ActivationFunctionType

    kc_n = d // 128          # contraction chunks for projections
    nb_n = s // 512          # 512-token blocks
    qt_n = s // 512          # q tiles (512 wide) in attention
    kb_n = s // 128          # k blocks (128 wide)

    nc = bacc.Bacc("TRN2", target_bir_lowering=False, debug=False)

    xT_d = nc.dram_tensor("xT", (d, s), bf16, kind="ExternalInput")
    wq_d = nc.dram_tensor("wq", (d, nhl * hd), bf16, kind="ExternalInput")
    wk_d = nc.dram_tensor("wk", (d, hd), bf16, kind="ExternalInput")
    wv_d = nc.dram_tensor("wv", (d, hd), bf16, kind="ExternalInput")
    wo_d = nc.dram_tensor("wo", (nhl * hd, d), bf16, kind="ExternalInput")
    m1q_d = nc.dram_tensor("m1q", (hd, s), f32, kind="ExternalInput")
    m2q_d = nc.dram_tensor("m2q", (hd, s), f32, kind="ExternalInput")
    m1k_d = nc.dram_tensor("m1k", (hd, s), f32, kind="ExternalInput")
    m2k_d = nc.dram_tensor("m2k", (hd, s), f32, kind="ExternalInput")
    tri_d = nc.dram_tensor("tri", (128, 128), bf16, kind="ExternalInput")
    y_d = nc.dram_tensor("y", (s, d), f32, kind="ExternalOutput")

    with tile.TileContext(nc) as tc, nc.allow_low_precision(
        reason="bf16 compute by design; fp32 accumulation in PSUM"
    ):
        with (
            tc.tile_pool(name="const", bufs=1) as const,
            tc.tile_pool(name="persist", bufs=1) as persist,
        ):
            # ---- resident weights / coefficients -------------------------
            wq_sb = persist.tile([128, kc_n, nhl * hd], bf16, tag="wq")
            wq_re = wq_d.rearrange("(kc p) m -> p kc m", p=128)
            for kc in range(kc_n):
                nc.sync.dma_start(wq_sb[:, kc, :], wq_re[:, kc, :])
            wk_sb = persist.tile([128, kc_n, hd], bf16, tag="wk")
            wk_re = wk_d.rearrange("(kc p) m -> p kc m", p=128)
            wv_sb = persist.tile([128, kc_n, hd], bf16, tag="wv")
            wv_re = wv_d.rearrange("(kc p) m -> p kc m", p=128)
            for kc in range(kc_n):
                nc.sync.dma_start(wk_sb[:, kc, :], wk_re[:, kc, :])
                nc.sync.dma_start(wv_sb[:, kc, :], wv_re[:, kc, :])
            wo_sb = persist.tile([128, nhl, d], bf16, tag="wo")
            wo_re = wo_d.rearrange("(h p) m -> p h m", p=128)
            for h in range(nhl):
                nc.sync.dma_start(wo_sb[:, h, :], wo_re[:, h, :])

            m1q_sb = persist.tile([hd, s], f32, tag="m1q")
            m2q_sb = persist.tile([hd, s], f32, tag="m2q")
            m1k_sb = persist.tile([hd, s], f32, tag="m1k")
            m2k_sb = persist.tile([hd, s], f32, tag="m2k")
            nc.sync.dma_start(m1q_sb[:], m1q_d[:])
            nc.sync.dma_start(m2q_sb[:], m2q_d[:])
            nc.sync.dma_start(m1k_sb[:], m1k_d[:])
            nc.sync.dma_start(m2k_sb[:], m2k_d[:])
            tri_sb = const.tile([128, 128], bf16, tag="tri")
            nc.sync.dma_start(tri_sb[:], tri_d[:])

            ones_k = const.tile([128, 1], bf16, tag="ones_k")
            nc.vector.memset(ones_k[:], 1.0)
            ones_1 = const.tile([1, 128], bf16, tag="ones_1")
            nc.vector.memset(ones_1[:], 1.0)
            eps_sb = const.tile([128, 1], f32, tag="eps")
            nc.vector.memset(eps_sb[:], EPS)
            ident = const.tile([128, 128], bf16, tag="ident")
            from concourse.masks import make_identity
            make_identity(nc, ident[:])

            # ---- persistent activations ---------------------------------
            qT_sb = [persist.tile([hd, s], bf16, tag=f"qT{h}", name=f"qT{h}") for h in range(nhl)]
            kT_sb = persist.tile([hd, s], bf16, tag="kT")
            v_sb = persist.tile([128, kb_n, hd + 1], bf16, tag="v")
            nc.vector.memset(v_sb[:, :, hd:hd + 1], 1.0)
            attT_sb = [persist.tile([hd, s], bf16, tag=f"attT{h}", name=f"attT{h}") for h in range(nhl)]

            # ================= Phase A: projections + norm + rope =========
            with (
                tc.tile_pool(name="xtp", bufs=2) as xtp,
                tc.tile_pool(name="workA", bufs=3) as wa,
                tc.tile_pool(name="psA", bufs=2, space="PSUM") as psA,
            ):
                xT_re = xT_d.rearrange("(kc p) n -> p kc n", p=128)

                def norm_rope_chain(q_ps, t, cs):
                    # rmsnorm via ones-matmul + bcast-matmul, rope via coeff
                    # tiles; PE ops here are emitted one tensor late so the
                    # ACT/DVE chain overlaps the next tensor's projection MMs.
                    sq = wa.tile([128, 512], bf16, tag="sq", name="sq")
                    nc.scalar.activation(sq[:], q_ps[:], AF.Square)
                    ssq = psA.tile([1, 512], f32, tag="ssq", name="ssq", bufs=1)
                    nc.tensor.matmul(ssq[:], ones_k[:], sq[:])
                    ssq_bf = wa.tile([1, 512], bf16, tag="ssq_bf", name="ssq_bf")
                    nc.scalar.copy(ssq_bf[:], ssq[:])
                    rb_ps = psA.tile([128, 512], f32, tag="rb_ps", name="rb_ps", bufs=1)
                    nc.tensor.matmul(rb_ps[:], ones_1[:], ssq_bf[:])
                    tmp = wa.tile([128, 512], f32, tag="tmp", name="tmp")
                    nc.scalar.activation(
                        tmp[:], rb_ps[:], AF.Sqrt, scale=1.0 / hd, bias=eps_sb[:]
                    )
                    rb = wa.tile([128, 512], f32, tag="rb", name="rb")
                    nc.vector.reciprocal_approx_fast(rb[:], tmp[:])
                    qn = wa.tile([128, 512], f32, tag="qn", name="qn")
                    nc.vector.tensor_mul(qn[:], q_ps[:], rb[:])
                    qs = wa.tile([128, 512], f32, tag="qs", name="qs")
                    nc.sync.dma_start(qs[0:64, :], qn[64:128, :])
                    nc.sync.dma_start(qs[64:128, :], qn[0:64, :])
                    m1 = m1q_sb if t < nhl else m1k_sb
                    m2 = m2q_sb if t < nhl else m2k_sb
                    t1 = wa.tile([128, 512], f32, tag="t1", name="t1")
                    nc.vector.tensor_mul(t1[:], qn[:], m1[:, cs])
                    t2 = wa.tile([128, 512], f32, tag="t2", name="t2")
                    nc.vector.tensor_mul(t2[:], qs[:], m2[:, cs])
                    dest = qT_sb[t] if t < nhl else kT_sb
                    nc.vector.tensor_add(dest[:, cs], t1[:], t2[:])

                pending = None
                for nb in range(nb_n):
                    cs = slice(nb * 512, (nb + 1) * 512)
                    xt = xtp.tile([128, kc_n, 512], bf16, tag="xt")
                    for kc in range(kc_n):
                        nc.sync.dma_start(xt[:, kc, :], xT_re[:, kc, cs])

                    # q heads then k: projection MMs now, norm chain deferred
                    for t in list(range(nhl)) + [nhl]:
                        q_ps = psA.tile([128, 512], f32, tag="q_ps", bufs=3)
                        for kc in range(kc_n):
                            if t < nhl:
                                lhsT = wq_sb[:, kc, t * hd:(t + 1) * hd]
                            else:
                                lhsT = wk_sb[:, kc, :]
                            nc.tensor.matmul(
                                q_ps[:], lhsT, xt[:, kc, :],
                                start=(kc == 0), stop=(kc == kc_n - 1),
                            )
                        if pending is not None:
                            norm_rope_chain(*pending)
                        pending = (q_ps, t, cs)

                    # v: plain projection, token-major
                    for tt in range(4):
                        v_ps = psA.tile([128, hd], f32, tag="v_ps", bufs=2)
                        for kc in range(kc_n):
                            nc.tensor.matmul(
                                v_ps[:],
                                xt[:, kc, tt * 128:(tt + 1) * 128],
                                wv_sb[:, kc, :],
                                start=(kc == 0), stop=(kc == kc_n - 1),
                            )
                        nc.vector.tensor_copy(v_sb[:, nb * 4 + tt, 0:hd], v_ps[:])
                if pending is not None:
                    norm_rope_chain(*pending)

            # ================= Phase B: causal flash attention ============
            with (
                tc.tile_pool(name="workB", bufs=3) as wb,
                tc.tile_pool(name="psB", bufs=2, space="PSUM") as psB,
            ):
                for h in range(nhl):
                    for qt in range(qt_n):
                        qcs = slice(qt * 512, (qt + 1) * 512)
                        nkb = 4 * qt + 4
                        att_ps = [
                            psB.tile([128, hd + 1], f32, tag="att", bufs=4,
                                     name=f"att{qs}")
                            for qs in range(4)
                        ]
                        s_tiles = {}

                        def emit_s(kb):
                            sp = psB.tile([128, 512], f32, tag="s_ps", name="s_ps", bufs=3)
                            r = kb - 4 * qt
                            c0 = 128 * r if r > 0 else 0
                            nc.tensor.matmul(
                                sp[:, c0:512],
                                kT_sb[:, kb * 128:(kb + 1) * 128],
                                qT_sb[h][:, qt * 512 + c0:(qt + 1) * 512],
                            )
                            s_tiles[kb] = sp

                        emit_s(0)
                        if nkb > 1:
                            emit_s(1)
                        for kb in range(nkb):
                            if kb + 2 < nkb:
                                emit_s(kb + 2)
                            sp = s_tiles.pop(kb)
                            p = wb.tile([128, 512], bf16, tag="p")
                            r = kb - 4 * qt
                            if r >= 0:
                                nc.scalar.activation(
                                    p[:, 128 * r:512], sp[:, 128 * r:512],
                                    AF.Exp, scale=SCALE,
                                )
                                nc.vector.tensor_mul(
                                    p[:, 128 * r:128 * (r + 1)],
                                    p[:, 128 * r:128 * (r + 1)],
                                    tri_sb[:],
                                )
                            else:
                                nc.scalar.activation(p[:], sp[:], AF.Exp, scale=SCALE)
                            for qs in range(4):
                                kmax = 4 * qt + qs
                                if kb > kmax:
                                    continue
                                nc.tensor.matmul(
                                    att_ps[qs][:],
                                    p[:, qs * 128:(qs + 1) * 128],
                                    v_sb[:, kb, :],
                                    start=(kb == 0), stop=(kb == kmax),
                                )
                        for qs in range(4):
                            rec = wb.tile([128, 1], f32, tag="rec")
                            nc.vector.reciprocal(rec[:], att_ps[qs][:, hd:hd + 1])
                            att_n = wb.tile([128, 128], bf16, tag="att_n")
                            nc.vector.tensor_scalar_mul(
                                att_n[:], att_ps[qs][:, 0:hd], rec[:]
                            )
                            tp_ps = psB.tile([128, 128], bf16, tag="tp", bufs=1)
                            nc.tensor.transpose(tp_ps[:], att_n[:], ident[:])
                            nc.vector.tensor_copy(
                                attT_sb[h][:, qt * 512 + qs * 128:
                                           qt * 512 + (qs + 1) * 128],
                                tp_ps[:],
                            )

            # ================= Phase C: output projection =================
            with (
                tc.tile_pool(name="workC", bufs=3) as wc,
                tc.tile_pool(name="psC", bufs=2, space="PSUM") as psC,
            ):
                for tt in range(s // 128):
                    for db in range(d // 512):
                        y_ps = psC.tile([128, 512], f32, tag="y")
                        for h in range(nhl):
                            nc.tensor.matmul(
                                y_ps[:],
                                attT_sb[h][:, tt * 128:(tt + 1) * 128],
                                wo_sb[:, h, db * 512:(db + 1) * 512],
                                start=(h == 0), stop=(h == nhl - 1),
                            )
                        y_sb = wc.tile([128, 512], f32, tag="ysb", name="ysb")
                        nc.scalar.copy(y_sb[:], y_ps[:])
                        nc.sync.dma_start(
                            y_d[tt * 128:(tt + 1) * 128, db * 512:(db + 1) * 512],
                            y_sb[:],
                        )

    nc.compile()
    return nc


def _rope_coeffs(norm_w, s=S, hd=HD):
    """Coefficient tiles [hd, s] folding rope cos/sin + permuted norm weight."""
    perm = np.concatenate([np.arange(0, hd, 2), np.arange(1, hd, 2)])
    w = np.asarray(norm_w, np.float64)[perm]
    half = hd // 2
    pos = np.arange(s, dtype=np.float64)
    inv_freq = 1.0 / (THETA ** (np.arange(0, hd, 2, dtype=np.float64) / hd))
    ang = pos[None, :] * inv_freq[:, None]          # [half, s]
    cos, sin = np.cos(ang), np.sin(ang)
    m1 = np.empty((hd, s), np.float32)
    m2 = np.empty((hd, s), np.float32)
    m1[:half] = cos * w[:half, None]
    m1[half:] = cos * w[half:, None]
    m2[:half] = -sin * w[half:, None]
    m2[half:] = sin * w[:half, None]
    return m1, m2


def _host_prep(x, wq, wk, wv, wo, q_norm_w, k_norm_w):
    perm = np.concatenate([np.arange(0, HD, 2), np.arange(1, HD, 2)])
    m1q, m2q = _rope_coeffs(q_norm_w)
    m1k, m2k = _rope_coeffs(k_norm_w)
    tri = np.triu(np.ones((128, 128), np.float32)).astype(BF16)

    in_maps = []
    for c in range(8):
        b, g = c // 4, c % 4
        heads = range(NHL * g, NHL * g + NHL)
        wq_loc = np.concatenate(
            [wq[:, h * HD:(h + 1) * HD][:, perm] for h in heads], axis=1
        )
        in_maps.append({
            "xT": np.ascontiguousarray(x[b].T).astype(BF16),
            "wq": np.ascontiguousarray(wq_loc).astype(BF16),
            "wk": np.ascontiguousarray(wk[:, g * HD:(g + 1) * HD][:, perm]).astype(BF16),
            "wv": np.ascontiguousarray(wv[:, g * HD:(g + 1) * HD]).astype(BF16),
            "wo": np.ascontiguousarray(wo[NHL * g * HD:NHL * (g + 1) * HD, :]).astype(BF16),
            "m1q": m1q, "m2q": m2q, "m1k": m1k, "m2k": m2k,
            "tri": tri,
        })
    return in_maps


def _install_ntff_shim():
    import types
    if "antenv.axon_hooks" in sys.modules:
        return
    mod = types.ModuleType("antenv.axon_hooks")
    _hook = [None]
    mod.set_axon_ntff_profile_hook = lambda h: _hook.__setitem__(0, h)
    mod.get_axon_ntff_profile_hook = lambda: _hook[0]
    sys.modules["antenv.axon_hooks"] = mod
    try:
        from trn_agent_boot.trn_boot import _ntff_profile_via_ctypes
        mod.set_axon_ntff_profile_hook(
            _ntff_profile_via_ctypes("/opt/axon/libaxon_pjrt.so")
        )
    except Exception:
        pass


LAST_EXEC_NS = None


def kernel(x, wq, wk, wv, wo, q_norm_w, k_norm_w):
    global LAST_EXEC_NS
    from concourse import bass_utils

    x = np.asarray(x)
    if "nc" not in _CACHED:
        _CACHED["nc"] = build_nc()
    nc = _CACHED["nc"]

    in_maps = _host_prep(
        np.asarray(x, np.float32), np.asarray(wq, np.float32),
        np.asarray(wk, np.float32), np.asarray(wv, np.float32),
        np.asarray(wo, np.float32), np.asarray(q_norm_w, np.float32),
        np.asarray(k_norm_w, np.float32),
    )
    trace = bool(int(os.environ.get("BASS_KERNEL_TRACE", "0")))
    if trace:
        _install_ntff_shim()
    res = bass_utils.run_bass_kernel_spmd(
        nc, in_maps, core_ids=list(range(8)), trace=trace
    )
    LAST_EXEC_NS = res.exec_time_ns
    y = np.zeros((B, S, D), np.float32)
    for c in range(8):
        y[c // 4] += res.results[c]["y"]
    return y


# revision 20
# speedup vs baseline: 1.1009x; 1.0552x over previous
"""Trainium2 Bass kernel for GQA attention block (nn_Attention_81372450390110).

Module: y = AttnOut(x) with q/k RMSNorm + interleaved RoPE + causal GQA
(NH=16 q heads, KVH=4 kv heads, HD=128, D=2048, B=2, S=2048).

Sharding: 8 cores = 2 batches x 4 KV groups. Core c handles batch c//4 and
KV group c%4 (4 q heads + 1 kv head). Each core computes a full [S, D]
partial of the output projection (row-parallel over heads); the host sums
the 4 group-partials per batch.

Layout strategy (all feature-major, "transposed"):
  - host passes xT = x[b].T so the D contraction dim lands on partitions
  - qT/kT computed as [HD, S] directly (lhsT = weight chunk)
  - scores computed transposed: sT[k, q] = kT_blk.T @ qT_blk
  - softmax without max-subtraction (rmsnorm bounds |scores| <= sqrt(HD))
  - P@V untransposed (lhsT = P chunk, rhs = v block augmented with a ones
    column) -> out [q, HD | l]: the softmax denominator l falls out as the
    129th column, normalized via per-partition tensor_scalar, then the
    [128,128] tile is PE-transposed into attn_outT for the o-projection
  - cross-partition sums/broadcasts via ones-matmuls on TensorE
  - RoPE+norm-weight folded into host-precomputed coefficient tiles, with
    an even/odd deinterleaving permutation baked into wq/wk columns
"""

import os
import sys

sys.path.insert(0, "/opt/trn_rl_repo")

import numpy as np
import ml_dtypes

BF16 = ml_dtypes.bfloat16

B = 2
S = 2048
D = 2048
NH = 16
KVH = 4
HD = 128
THETA = 10000.0
EPS = 1e-6
NHL = NH // KVH  # q heads per core (4)
SCALE = 1.0 / float(np.sqrt(HD))

_CACHED = {}


def build_nc(s=S, d=D, nhl=NHL, hd=HD):
    import concourse.mybir as mybir
    import concourse.tile as tile
    from concourse import bacc

    f32 = mybir.dt.float32
    bf16 = mybir.dt.bfloat16
    AF = mybir.ActivationFunctionType

    kc_n = d // 128          # contraction chunks for projections
    nb_n = s // 512          # 512-token blocks
    qt_n = s // 512          # q tiles (512 wide) in attention
    kb_n = s // 128          # k blocks (128 wide)

    nc = bacc.Bacc("TRN2", target_bir_lowering=False, debug=False)

    xT_d = nc.dram_tensor("xT", (d, s), bf16, kind="ExternalInput")
    wq_d = nc.dram_tensor("wq", (d, nhl * hd), bf16, kind="ExternalInput")
    wk_d = nc.dram_tensor("wk", (d, hd), bf16, kind="ExternalInput")
    wv_d = nc.dram_tensor("wv", (d, hd), bf16, kind="ExternalInput")
    wo_d = nc.dram_tensor("wo", (nhl * hd, d), bf16, kind="ExternalInput")
    m1q_d = nc.dram_tensor("m1q", (hd, s), bf16, kind="ExternalInput")
    m2q_d = nc.dram_tensor("m2q", (hd, s), bf16, kind="ExternalInput")
    m1k_d = nc.dram_tensor("m1k", (hd, s), bf16, kind="ExternalInput")
    m2k_d = nc.dram_tensor("m2k", (hd, s), bf16, kind="ExternalInput")
    tri_d = nc.dram_tensor("tri", (128, 128), bf16, kind="ExternalInput")
    y_d = nc.dram_tensor("y", (s, d), f32, kind="ExternalOutput")

    with tile.TileContext(nc) as tc, nc.allow_low_precision(
        reason="bf16 compute by design; fp32 accumulation in PSUM"
    ):
        with (
            tc.tile_pool(name="const", bufs=1) as const,
            tc.tile_pool(name="persist", bufs=1) as persist,
        ):
            # ---- resident weights / coefficients -------------------------
            wq_sb = persist.tile([128, kc_n, nhl * hd], bf16, tag="wq")
            wq_re = wq_d.rearrange("(kc p) m -> p kc m", p=128)
            for kc in range(kc_n):
                nc.sync.dma_start(wq_sb[:, kc, :], wq_re[:, kc, :])
            wk_sb = persist.tile([128, kc_n, hd], bf16, tag="wk")
            wk_re = wk_d.rearrange("(kc p) m -> p kc m", p=128)
            wv_sb = persist.tile([128, kc_n, hd], bf16, tag="wv")
            wv_re = wv_d.rearrange("(kc p) m -> p kc m", p=128)
            for kc in range(kc_n):
                nc.sync.dma_start(wk_sb[:, kc, :], wk_re[:, kc, :])
                nc.sync.dma_start(wv_sb[:, kc, :], wv_re[:, kc, :])
            wo_sb = persist.tile([128, nhl, d], bf16, tag="wo")
            wo_re = wo_d.rearrange("(h p) m -> p h m", p=128)

            m1q_sb = persist.tile([hd, s], bf16, tag="m1q")
            m2q_sb = persist.tile([hd, s], bf16, tag="m2q")
            m1k_sb = persist.tile([hd, s], bf16, tag="m1k")
            m2k_sb = persist.tile([hd, s], bf16, tag="m2k")
            tri_sb = const.tile([128, 128], bf16, tag="tri")
            nc.sync.dma_start(tri_sb[:], tri_d[:])

            ones_k = const.tile([128, 1], bf16, tag="ones_k")
            nc.vector.memset(ones_k[:], 1.0)
            ones_1 = const.tile([1, 128], bf16, tag="ones_1")
            nc.vector.memset(ones_1[:], 1.0)
            eps_sb = const.tile([128, 1], f32, tag="eps")
            nc.vector.memset(eps_sb[:], EPS)
            ident = const.tile([128, 128], bf16, tag="ident")
            from concourse.masks import make_identity
            make_identity(nc, ident[:])

            # ---- persistent activations ---------------------------------
            qT_sb = [persist.tile([hd, s], bf16, tag=f"qT{h}", name=f"qT{h}") for h in range(nhl)]
            kT_sb = persist.tile([hd, s], bf16, tag="kT")
            v_sb = persist.tile([128, kb_n, hd + 1], bf16, tag="v")
            nc.vector.memset(v_sb[:, :, hd:hd + 1], 1.0)
            attT_sb = [persist.tile([hd, s], bf16, tag=f"attT{h}", name=f"attT{h}") for h in range(nhl)]

            # ================= Phase A: projections + norm + rope =========
            with (
                tc.tile_pool(name="xtp", bufs=2) as xtp,
                tc.tile_pool(name="workA", bufs=3) as wa,
                tc.tile_pool(name="psA", bufs=2, space="PSUM") as psA,
            ):
                xT_re = xT_d.rearrange("(kc p) n -> p kc n", p=128)

                def norm_rope_chain(q_ps, t, cs):
                    # rmsnorm via ones-matmul + bcast-matmul, rope via coeff
                    # tiles; PE ops here are emitted one tensor late so the
                    # ACT/DVE chain overlaps the next tensor's projection MMs.
                    sq = wa.tile([128, 512], bf16, tag="sq", name="sq")
                    nc.scalar.activation(sq[:], q_ps[:], AF.Square)
                    ssq = psA.tile([1, 512], f32, tag="ssq", name="ssq", bufs=1)
                    nc.tensor.matmul(ssq[:], ones_k[:], sq[:])
                    ssq_bf = wa.tile([1, 512], bf16, tag="ssq_bf", name="ssq_bf")
                    nc.scalar.copy(ssq_bf[:], ssq[:])
                    rb_ps = psA.tile([128, 512], f32, tag="rb_ps", name="rb_ps", bufs=1)
                    nc.tensor.matmul(rb_ps[:], ones_1[:], ssq_bf[:])
                    tmp = wa.tile([128, 512], f32, tag="tmp", name="tmp")
                    nc.scalar.activation(
                        tmp[:], rb_ps[:], AF.Sqrt, scale=1.0 / hd, bias=eps_sb[:]
                    )
                    rb = wa.tile([128, 512], f32, tag="rb", name="rb")
                    nc.vector.reciprocal_approx_fast(rb[:], tmp[:])
                    qn = wa.tile([128, 512], f32, tag="qn", name="qn")
                    nc.vector.tensor_mul(qn[:], q_ps[:], rb[:])
                    qs = wa.tile([128, 512], f32, tag="qs", name="qs")
                    nc.sync.dma_start(qs[0:64, :], qn[64:128, :])
                    nc.sync.dma_start(qs[64:128, :], qn[0:64, :])
                    m1 = m1q_sb if t < nhl else m1k_sb
                    m2 = m2q_sb if t < nhl else m2k_sb
                    t1 = wa.tile([128, 512], f32, tag="t1", name="t1")
                    nc.vector.tensor_mul(t1[:], qn[:], m1[:, cs])
                    t2 = wa.tile([128, 512], f32, tag="t2", name="t2")
                    nc.vector.tensor_mul(t2[:], qs[:], m2[:, cs])
                    dest = qT_sb[t] if t < nhl else kT_sb
                    nc.vector.tensor_add(dest[:, cs], t1[:], t2[:])

                pending = None
                for nb in range(nb_n):
                    cs = slice(nb * 512, (nb + 1) * 512)
                    xt = xtp.tile([128, kc_n, 512], bf16, tag="xt")
                    for kc in range(kc_n):
                        nc.sync.dma_start(xt[:, kc, :], xT_re[:, kc, cs])
                    if nb == 0:
                        nc.sync.dma_start(m1q_sb[:], m1q_d[:])
                        nc.sync.dma_start(m2q_sb[:], m2q_d[:])
                        nc.sync.dma_start(m1k_sb[:], m1k_d[:])
                        nc.sync.dma_start(m2k_sb[:], m2k_d[:])

                    # q heads then k: projection MMs now, norm chain deferred
                    for t in list(range(nhl)) + [nhl]:
                        q_ps = psA.tile([128, 512], f32, tag="q_ps", bufs=3)
                        for kc in range(kc_n):
                            if t < nhl:
                                lhsT = wq_sb[:, kc, t * hd:(t + 1) * hd]
                            else:
                                lhsT = wk_sb[:, kc, :]
                            nc.tensor.matmul(
                                q_ps[:], lhsT, xt[:, kc, :],
                                start=(kc == 0), stop=(kc == kc_n - 1),
                            )
                        if pending is not None:
                            norm_rope_chain(*pending)
                        pending = (q_ps, t, cs)

                    # v: plain projection, token-major
                    for tt in range(4):
                        v_ps = psA.tile([128, hd], f32, tag="v_ps", bufs=2)
                        for kc in range(kc_n):
                            nc.tensor.matmul(
                                v_ps[:],
                                xt[:, kc, tt * 128:(tt + 1) * 128],
                                wv_sb[:, kc, :],
                                start=(kc == 0), stop=(kc == kc_n - 1),
                            )
                        nc.vector.tensor_copy(v_sb[:, nb * 4 + tt, 0:hd], v_ps[:])
                if pending is not None:
                    norm_rope_chain(*pending)

            # ================= Phase B: causal flash attention ============
            with (
                tc.tile_pool(name="workB", bufs=3) as wb,
                tc.tile_pool(name="psB", bufs=2, space="PSUM") as psB,
            ):
                for h in range(nhl):
                    for qt in range(qt_n):
                        qcs = slice(qt * 512, (qt + 1) * 512)
                        nkb = 4 * qt + 4
                        att_ps = [
                            psB.tile([128, hd + 1], f32, tag="att", bufs=4,
                                     name=f"att{qs}")
                            for qs in range(4)
                        ]
                        s_tiles = {}

                        def emit_s(kb):
                            sp = psB.tile([128, 512], f32, tag="s_ps", name="s_ps", bufs=3)
                            r = kb - 4 * qt
                            c0 = 128 * r if r > 0 else 0
                            nc.tensor.matmul(
                                sp[:, c0:512],
                                kT_sb[:, kb * 128:(kb + 1) * 128],
                                qT_sb[h][:, qt * 512 + c0:(qt + 1) * 512],
                            )
                            s_tiles[kb] = sp

                        emit_s(0)
                        if nkb > 1:
                            emit_s(1)
                        for kb in range(nkb):
                            if kb + 2 < nkb:
                                emit_s(kb + 2)
                            sp = s_tiles.pop(kb)
                            p = wb.tile([128, 512], bf16, tag="p")
                            r = kb - 4 * qt
                            if r >= 0:
                                nc.scalar.activation(
                                    p[:, 128 * r:512], sp[:, 128 * r:512],
                                    AF.Exp, scale=SCALE,
                                )
                                nc.vector.tensor_mul(
                                    p[:, 128 * r:128 * (r + 1)],
                                    p[:, 128 * r:128 * (r + 1)],
                                    tri_sb[:],
                                )
                            else:
                                nc.scalar.activation(p[:], sp[:], AF.Exp, scale=SCALE)
                            for qs in range(4):
                                kmax = 4 * qt + qs
                                if kb > kmax:
                                    continue
                                nc.tensor.matmul(
                                    att_ps[qs][:],
                                    p[:, qs * 128:(qs + 1) * 128],
                                    v_sb[:, kb, :],
                                    start=(kb == 0), stop=(kb == kmax),
                                )
                        for qs in range(4):
                            rec = wb.tile([128, 1], f32, tag="rec")
                            nc.vector.reciprocal(rec[:], att_ps[qs][:, hd:hd + 1])
                            att_n = wb.tile([128, 128], bf16, tag="att_n")
                            nc.vector.tensor_scalar_mul(
                                att_n[:], att_ps[qs][:, 0:hd], rec[:]
                            )
                            tp_ps = psB.tile([128, 128], bf16, tag="tp", bufs=1)
                            nc.tensor.transpose(tp_ps[:], att_n[:], ident[:])
                            nc.vector.tensor_copy(
                                attT_sb[h][:, qt * 512 + qs * 128:
                                           qt * 512 + (qs + 1) * 128],
                                tp_ps[:],
                            )

            # ================= Phase C: output projection =================
            with (
                tc.tile_pool(name="workC", bufs=3) as wc,
                tc.tile_pool(name="psC", bufs=2, space="PSUM") as psC,
            ):
                for h in range(nhl):
                    nc.sync.dma_start(wo_sb[:, h, :], wo_re[:, h, :])
                for tt in range(s // 128):
                    for db in range(d // 512):
                        y_ps = psC.tile([128, 512], f32, tag="y")
                        for h in range(nhl):
                            nc.tensor.matmul(
                                y_ps[:],
                                attT_sb[h][:, tt * 128:(tt + 1) * 128],
                                wo_sb[:, h, db * 512:(db + 1) * 512],
                                start=(h == 0), stop=(h == nhl - 1),
                            )
                        y_sb = wc.tile([128, 512], f32, tag="ysb", name="ysb")
                        nc.scalar.copy(y_sb[:], y_ps[:])
                        nc.sync.dma_start(
                            y_d[tt * 128:(tt + 1) * 128, db * 512:(db + 1) * 512],
                            y_sb[:],
                        )

    nc.compile()
    return nc


def _rope_coeffs(norm_w, s=S, hd=HD):
    """Coefficient tiles [hd, s] folding rope cos/sin + permuted norm weight."""
    perm = np.concatenate([np.arange(0, hd, 2), np.arange(1, hd, 2)])
    w = np.asarray(norm_w, np.float64)[perm]
    half = hd // 2
    pos = np.arange(s, dtype=np.float64)
    inv_freq = 1.0 / (THETA ** (np.arange(0, hd, 2, dtype=np.float64) / hd))
    ang = pos[None, :] * inv_freq[:, None]          # [half, s]
    cos, sin = np.cos(ang), np.sin(ang)
    m1 = np.empty((hd, s), np.float32)
    m2 = np.empty((hd, s), np.float32)
    m1[:half] = cos * w[:half, None]
    m1[half:] = cos * w[half:, None]
    m2[:half] = -sin * w[half:, None]
    m2[half:] = sin * w[:half, None]
    return m1, m2


def _host_prep(x, wq, wk, wv, wo, q_norm_w, k_norm_w):
    perm = np.concatenate([np.arange(0, HD, 2), np.arange(1, HD, 2)])
    m1q, m2q = _rope_coeffs(q_norm_w)
    m1k, m2k = _rope_coeffs(k_norm_w)
    tri = np.triu(np.ones((128, 128), np.float32)).astype(BF16)

    in_maps = []
    for c in range(8):
        b, g = c // 4, c % 4
        heads = range(NHL * g, NHL * g + NHL)
        wq_loc = np.concatenate(
            [wq[:, h * HD:(h + 1) * HD][:, perm] for h in heads], axis=1
        )
        in_maps.append({
            "xT": np.ascontiguousarray(x[b].T).astype(BF16),
            "wq": np.ascontiguousarray(wq_loc).astype(BF16),
            "wk": np.ascontiguousarray(wk[:, g * HD:(g + 1) * HD][:, perm]).astype(BF16),
            "wv": np.ascontiguousarray(wv[:, g * HD:(g + 1) * HD]).astype(BF16),
            "wo": np.ascontiguousarray(wo[NHL * g * HD:NHL * (g + 1) * HD, :]).astype(BF16),
            "m1q": m1q.astype(BF16), "m2q": m2q.astype(BF16),
            "m1k": m1k.astype(BF16), "m2k": m2k.astype(BF16),
            "tri": tri,
        })
    return in_maps


def _install_ntff_shim():
    import types
    if "antenv.axon_hooks" in sys.modules:
        return
    mod = types.ModuleType("antenv.axon_hooks")
    _hook = [None]
    mod.set_axon_ntff_profile_hook = lambda h: _hook.__setitem__(0, h)
    mod.get_axon_ntff_profile_hook = lambda: _hook[0]
    sys.modules["antenv.axon_hooks"] = mod
    try:
        from trn_agent_boot.trn_boot import _ntff_profile_via_ctypes
        mod.set_axon_ntff_profile_hook(
            _ntff_profile_via_ctypes("/opt/axon/libaxon_pjrt.so")
        )
    except Exception:
        pass


LAST_EXEC_NS = None


def kernel(x, wq, wk, wv, wo, q_norm_w, k_norm_w):
    global LAST_EXEC_NS
    from concourse import bass_utils

    x = np.asarray(x)
    if "nc" not in _CACHED:
        _CACHED["nc"] = build_nc()
    nc = _CACHED["nc"]

    in_maps = _host_prep(
        np.asarray(x, np.float32), np.asarray(wq, np.float32),
        np.asarray(wk, np.float32), np.asarray(wv, np.float32),
        np.asarray(wo, np.float32), np.asarray(q_norm_w, np.float32),
        np.asarray(k_norm_w, np.float32),
    )
    trace = bool(int(os.environ.get("BASS_KERNEL_TRACE", "0")))
    if trace:
        _install_ntff_shim()
    res = bass_utils.run_bass_kernel_spmd(
        nc, in_maps, core_ids=list(range(8)), trace=trace
    )
    LAST_EXEC_NS = res.exec_time_ns
    y = np.zeros((B, S, D), np.float32)
    for c in range(8):
        y[c // 4] += res.results[c]["y"]
    return y


# revision 24
# speedup vs baseline: 1.1266x; 1.0233x over previous
"""Trainium2 Bass kernel for GQA attention block (nn_Attention_81372450390110).

Module: y = AttnOut(x) with q/k RMSNorm + interleaved RoPE + causal GQA
(NH=16 q heads, KVH=4 kv heads, HD=128, D=2048, B=2, S=2048).

Sharding: 8 cores = 2 batches x 4 KV groups. Core c handles batch c//4 and
KV group c%4 (4 q heads + 1 kv head). Each core computes a full [S, D]
partial of the output projection (row-parallel over heads); the host sums
the 4 group-partials per batch.

Layout strategy (all feature-major, "transposed"):
  - host passes xT = x[b].T so the D contraction dim lands on partitions
  - qT/kT computed as [HD, S] directly (lhsT = weight chunk)
  - scores computed transposed: sT[k, q] = kT_blk.T @ qT_blk
  - softmax without max-subtraction (rmsnorm bounds |scores| <= sqrt(HD))
  - P@V untransposed (lhsT = P chunk, rhs = v block augmented with a ones
    column) -> out [q, HD | l]: the softmax denominator l falls out as the
    129th column, normalized via per-partition tensor_scalar, then the
    [128,128] tile is PE-transposed into attn_outT for the o-projection
  - cross-partition sums/broadcasts via ones-matmuls on TensorE
  - RoPE+norm-weight folded into host-precomputed coefficient tiles, with
    an even/odd deinterleaving permutation baked into wq/wk columns
"""

import os
import sys

sys.path.insert(0, "/opt/trn_rl_repo")

import numpy as np
import ml_dtypes

BF16 = ml_dtypes.bfloat16

B = 2
S = 2048
D = 2048
NH = 16
KVH = 4
HD = 128
THETA = 10000.0
EPS = 1e-6
NHL = NH // KVH  # q heads per core (4)
SCALE = 1.0 / float(np.sqrt(HD))

_CACHED = {}


def build_nc(s=S, d=D, nhl=NHL, hd=HD):
    import concourse.mybir as mybir
    import concourse.tile as tile
    from concourse import bacc

    f32 = mybir.dt.float32
    bf16 = mybir.dt.bfloat16
    AF = mybir.ActivationFunctionType

    kc_n = d // 128          # contraction chunks for projections
    nb_n = s // 512          # 512-token blocks
    qt_n = s // 512          # q tiles (512 wide) in attention
    kb_n = s // 128          # k blocks (128 wide)

    nc = bacc.Bacc("TRN2", target_bir_lowering=False, debug=False)

    xT_d = nc.dram_tensor("xT", (d, s), bf16, kind="ExternalInput")
    wq_d = nc.dram_tensor("wq", (d, nhl * hd), bf16, kind="ExternalInput")
    wk_d = nc.dram_tensor("wk", (d, hd), bf16, kind="ExternalInput")
    wv_d = nc.dram_tensor("wv", (d, hd), bf16, kind="ExternalInput")
    wo_d = nc.dram_tensor("wo", (nhl * hd, d), bf16, kind="ExternalInput")
    m1q_d = nc.dram_tensor("m1q", (hd, s), bf16, kind="ExternalInput")
    m2q_d = nc.dram_tensor("m2q", (hd, s), bf16, kind="ExternalInput")
    m1k_d = nc.dram_tensor("m1k", (hd, s), bf16, kind="ExternalInput")
    m2k_d = nc.dram_tensor("m2k", (hd, s), bf16, kind="ExternalInput")
    tri_d = nc.dram_tensor("tri", (128, 128), bf16, kind="ExternalInput")
    y_d = nc.dram_tensor("y", (s, d), f32, kind="ExternalOutput")

    with tile.TileContext(nc) as tc, nc.allow_low_precision(
        reason="bf16 compute by design; fp32 accumulation in PSUM"
    ):
        with (
            tc.tile_pool(name="const", bufs=1) as const,
            tc.tile_pool(name="persist", bufs=1) as persist,
        ):
            # ---- resident weights / coefficients -------------------------
            wq_sb = persist.tile([128, kc_n, nhl * hd], bf16, tag="wq")
            wq_re = wq_d.rearrange("(kc p) m -> p kc m", p=128)
            for kc in range(kc_n):
                nc.sync.dma_start(wq_sb[:, kc, :], wq_re[:, kc, :])
            wk_sb = persist.tile([128, kc_n, hd], bf16, tag="wk")
            wk_re = wk_d.rearrange("(kc p) m -> p kc m", p=128)
            wv_sb = persist.tile([128, kc_n, hd], bf16, tag="wv")
            wv_re = wv_d.rearrange("(kc p) m -> p kc m", p=128)
            for kc in range(kc_n):
                nc.sync.dma_start(wk_sb[:, kc, :], wk_re[:, kc, :])
                nc.sync.dma_start(wv_sb[:, kc, :], wv_re[:, kc, :])
            wo_sb = persist.tile([128, nhl, d], bf16, tag="wo")
            wo_re = wo_d.rearrange("(h p) m -> p h m", p=128)

            m1q_sb = persist.tile([hd, s], bf16, tag="m1q")
            m2q_sb = persist.tile([hd, s], bf16, tag="m2q")
            m1k_sb = persist.tile([hd, s], bf16, tag="m1k")
            m2k_sb = persist.tile([hd, s], bf16, tag="m2k")
            tri_sb = const.tile([128, 128], bf16, tag="tri")
            nc.sync.dma_start(tri_sb[:], tri_d[:])

            ones_k = const.tile([128, 1], bf16, tag="ones_k")
            nc.vector.memset(ones_k[:], 1.0)
            ones_1 = const.tile([1, 128], bf16, tag="ones_1")
            nc.vector.memset(ones_1[:], 1.0)
            eps_sb = const.tile([128, 1], f32, tag="eps")
            nc.vector.memset(eps_sb[:], EPS)
            ident = const.tile([128, 128], bf16, tag="ident")
            from concourse.masks import make_identity
            make_identity(nc, ident[:])

            # ---- persistent activations ---------------------------------
            qT_sb = [persist.tile([hd, s], bf16, tag=f"qT{h}", name=f"qT{h}") for h in range(nhl)]
            kT_sb = persist.tile([hd, s], bf16, tag="kT")
            v_sb = persist.tile([128, kb_n, hd + 1], bf16, tag="v")
            nc.vector.memset(v_sb[:, :, hd:hd + 1], 1.0)
            attT_sb = [persist.tile([hd, s], bf16, tag=f"attT{h}", name=f"attT{h}") for h in range(nhl)]

            # ================= Phase A: projections + norm + rope =========
            with (
                tc.tile_pool(name="xtp", bufs=2) as xtp,
                tc.tile_pool(name="workA", bufs=3) as wa,
                tc.tile_pool(name="psA", bufs=2, space="PSUM") as psA,
            ):
                xT_re = xT_d.rearrange("(kc p) n -> p kc n", p=128)

                def norm_rope_chain(q_ps, t, cs):
                    # rmsnorm via ones-matmul + bcast-matmul, rope via coeff
                    # tiles; PE ops here are emitted one tensor late so the
                    # ACT/DVE chain overlaps the next tensor's projection MMs.
                    sq = wa.tile([128, 512], bf16, tag="sq", name="sq")
                    nc.scalar.activation(sq[:], q_ps[:], AF.Square)
                    ssq = psA.tile([1, 512], f32, tag="ssq", name="ssq", bufs=1)
                    nc.tensor.matmul(ssq[:], ones_k[:], sq[:])
                    ssq_bf = wa.tile([1, 512], bf16, tag="ssq_bf", name="ssq_bf")
                    nc.scalar.copy(ssq_bf[:], ssq[:])
                    rb_ps = psA.tile([128, 512], f32, tag="rb_ps", name="rb_ps", bufs=1)
                    nc.tensor.matmul(rb_ps[:], ones_1[:], ssq_bf[:])
                    tmp = wa.tile([128, 512], f32, tag="tmp", name="tmp")
                    nc.scalar.activation(
                        tmp[:], rb_ps[:], AF.Sqrt, scale=1.0 / hd, bias=eps_sb[:]
                    )
                    rb = wa.tile([128, 512], f32, tag="rb", name="rb")
                    nc.vector.reciprocal_approx_fast(rb[:], tmp[:])
                    qn = wa.tile([128, 512], f32, tag="qn", name="qn")
                    nc.vector.tensor_mul(qn[:], q_ps[:], rb[:])
                    qs = wa.tile([128, 512], f32, tag="qs", name="qs")
                    nc.sync.dma_start(qs[0:64, :], qn[64:128, :])
                    nc.sync.dma_start(qs[64:128, :], qn[0:64, :])
                    m1 = m1q_sb if t < nhl else m1k_sb
                    m2 = m2q_sb if t < nhl else m2k_sb
                    t1 = wa.tile([128, 512], f32, tag="t1", name="t1")
                    nc.vector.tensor_mul(t1[:], qn[:], m1[:, cs])
                    t2 = wa.tile([128, 512], f32, tag="t2", name="t2")
                    nc.vector.tensor_mul(t2[:], qs[:], m2[:, cs])
                    dest = qT_sb[t] if t < nhl else kT_sb
                    nc.vector.tensor_add(dest[:, cs], t1[:], t2[:])

                pending = None
                for nb in range(nb_n):
                    cs = slice(nb * 512, (nb + 1) * 512)
                    xt = xtp.tile([128, kc_n, 512], bf16, tag="xt")
                    for kc in range(kc_n):
                        nc.sync.dma_start(xt[:, kc, :], xT_re[:, kc, cs])
                    if nb == 0:
                        nc.sync.dma_start(m1q_sb[:], m1q_d[:])
                        nc.sync.dma_start(m2q_sb[:], m2q_d[:])
                        nc.sync.dma_start(m1k_sb[:], m1k_d[:])
                        nc.sync.dma_start(m2k_sb[:], m2k_d[:])

                    # q heads then k: projection MMs now, norm chain deferred
                    for t in list(range(nhl)) + [nhl]:
                        q_ps = psA.tile([128, 512], f32, tag="q_ps", bufs=3)
                        for kc in range(kc_n):
                            if t < nhl:
                                lhsT = wq_sb[:, kc, t * hd:(t + 1) * hd]
                            else:
                                lhsT = wk_sb[:, kc, :]
                            nc.tensor.matmul(
                                q_ps[:], lhsT, xt[:, kc, :],
                                start=(kc == 0), stop=(kc == kc_n - 1),
                            )
                        if pending is not None:
                            norm_rope_chain(*pending)
                        pending = (q_ps, t, cs)

                    # v: plain projection, token-major
                    for tt in range(4):
                        v_ps = psA.tile([128, hd], f32, tag="v_ps", bufs=2)
                        for kc in range(kc_n):
                            nc.tensor.matmul(
                                v_ps[:],
                                xt[:, kc, tt * 128:(tt + 1) * 128],
                                wv_sb[:, kc, :],
                                start=(kc == 0), stop=(kc == kc_n - 1),
                            )
                        nc.vector.tensor_copy(v_sb[:, nb * 4 + tt, 0:hd], v_ps[:])
                if pending is not None:
                    norm_rope_chain(*pending)

            # ================= Phase B: causal flash attention ============
            with (
                tc.tile_pool(name="workB", bufs=3) as wb,
                tc.tile_pool(name="psB", bufs=2, space="PSUM") as psB,
            ):
                for h in range(nhl):
                    nc.sync.dma_start(wo_sb[:, h, :], wo_re[:, h, :])
                for qt in range(qt_n):
                    for h in range(nhl):
                        qcs = slice(qt * 512, (qt + 1) * 512)
                        nkb = 4 * qt + 4
                        att_ps = [
                            psB.tile([128, hd + 1], f32, tag="att", bufs=4,
                                     name=f"att{qs}")
                            for qs in range(4)
                        ]
                        s_tiles = {}

                        def emit_s(kb):
                            sp = psB.tile([128, 512], f32, tag="s_ps", name="s_ps", bufs=3)
                            r = kb - 4 * qt
                            c0 = 128 * r if r > 0 else 0
                            nc.tensor.matmul(
                                sp[:, c0:512],
                                kT_sb[:, kb * 128:(kb + 1) * 128],
                                qT_sb[h][:, qt * 512 + c0:(qt + 1) * 512],
                            )
                            s_tiles[kb] = sp

                        emit_s(0)
                        if nkb > 1:
                            emit_s(1)
                        for kb in range(nkb):
                            if kb + 2 < nkb:
                                emit_s(kb + 2)
                            sp = s_tiles.pop(kb)
                            p = wb.tile([128, 512], bf16, tag="p")
                            r = kb - 4 * qt
                            if r >= 0:
                                nc.scalar.activation(
                                    p[:, 128 * r:512], sp[:, 128 * r:512],
                                    AF.Exp, scale=SCALE,
                                )
                                nc.vector.tensor_mul(
                                    p[:, 128 * r:128 * (r + 1)],
                                    p[:, 128 * r:128 * (r + 1)],
                                    tri_sb[:],
                                )
                            else:
                                nc.scalar.activation(p[:], sp[:], AF.Exp, scale=SCALE)
                            for qs in range(4):
                                kmax = 4 * qt + qs
                                if kb > kmax:
                                    continue
                                nc.tensor.matmul(
                                    att_ps[qs][:],
                                    p[:, qs * 128:(qs + 1) * 128],
                                    v_sb[:, kb, :],
                                    start=(kb == 0), stop=(kb == kmax),
                                )
                        for qs in range(4):
                            rec = wb.tile([128, 1], f32, tag="rec")
                            nc.vector.reciprocal(rec[:], att_ps[qs][:, hd:hd + 1])
                            att_n = wb.tile([128, 128], bf16, tag="att_n")
                            nc.vector.tensor_scalar_mul(
                                att_n[:], att_ps[qs][:, 0:hd], rec[:]
                            )
                            tp_ps = psB.tile([128, 128], bf16, tag="tp", bufs=1)
                            nc.tensor.transpose(tp_ps[:], att_n[:], ident[:])
                            nc.vector.tensor_copy(
                                attT_sb[h][:, qt * 512 + qs * 128:
                                           qt * 512 + (qs + 1) * 128],
                                tp_ps[:],
                            )
                        if h == nhl - 1:
                            for tt in range(qt * 4, qt * 4 + 4):
                                for db in range(d // 512):
                                    y_ps = psB.tile(
                                        [128, 512], f32, tag="s_ps", name="y_ps",
                                        bufs=3,
                                    )
                                    for hh in range(nhl):
                                        nc.tensor.matmul(
                                            y_ps[:],
                                            attT_sb[hh][:, tt * 128:(tt + 1) * 128],
                                            wo_sb[:, hh, db * 512:(db + 1) * 512],
                                            start=(hh == 0), stop=(hh == nhl - 1),
                                        )
                                    y_sb = wb.tile([128, 512], f32, tag="ysb",
                                                   name="ysb")
                                    nc.scalar.copy(y_sb[:], y_ps[:])
                                    nc.sync.dma_start(
                                        y_d[tt * 128:(tt + 1) * 128,
                                            db * 512:(db + 1) * 512],
                                        y_sb[:],
                                    )



    nc.compile()
    return nc


def _rope_coeffs(norm_w, s=S, hd=HD):
    """Coefficient tiles [hd, s] folding rope cos/sin + permuted norm weight."""
    perm = np.concatenate([np.arange(0, hd, 2), np.arange(1, hd, 2)])
    w = np.asarray(norm_w, np.float64)[perm]
    half = hd // 2
    pos = np.arange(s, dtype=np.float64)
    inv_freq = 1.0 / (THETA ** (np.arange(0, hd, 2, dtype=np.float64) / hd))
    ang = pos[None, :] * inv_freq[:, None]          # [half, s]
    cos, sin = np.cos(ang), np.sin(ang)
    m1 = np.empty((hd, s), np.float32)
    m2 = np.empty((hd, s), np.float32)
    m1[:half] = cos * w[:half, None]
    m1[half:] = cos * w[half:, None]
    m2[:half] = -sin * w[half:, None]
    m2[half:] = sin * w[:half, None]
    return m1, m2


def _host_prep(x, wq, wk, wv, wo, q_norm_w, k_norm_w):
    perm = np.concatenate([np.arange(0, HD, 2), np.arange(1, HD, 2)])
    m1q, m2q = _rope_coeffs(q_norm_w)
    m1k, m2k = _rope_coeffs(k_norm_w)
    tri = np.triu(np.ones((128, 128), np.float32)).astype(BF16)

    in_maps = []
    for c in range(8):
        b, g = c // 4, c % 4
        heads = range(NHL * g, NHL * g + NHL)
        wq_loc = np.concatenate(
            [wq[:, h * HD:(h + 1) * HD][:, perm] for h in heads], axis=1
        )
        in_maps.append({
            "xT": np.ascontiguousarray(x[b].T).astype(BF16),
            "wq": np.ascontiguousarray(wq_loc).astype(BF16),
            "wk": np.ascontiguousarray(wk[:, g * HD:(g + 1) * HD][:, perm]).astype(BF16),
            "wv": np.ascontiguousarray(wv[:, g * HD:(g + 1) * HD]).astype(BF16),
            "wo": np.ascontiguousarray(wo[NHL * g * HD:NHL * (g + 1) * HD, :]).astype(BF16),
            "m1q": m1q.astype(BF16), "m2q": m2q.astype(BF16),
            "m1k": m1k.astype(BF16), "m2k": m2k.astype(BF16),
            "tri": tri,
        })
    return in_maps


def _install_ntff_shim():
    import types
    if "antenv.axon_hooks" in sys.modules:
        return
    mod = types.ModuleType("antenv.axon_hooks")
    _hook = [None]
    mod.set_axon_ntff_profile_hook = lambda h: _hook.__setitem__(0, h)
    mod.get_axon_ntff_profile_hook = lambda: _hook[0]
    sys.modules["antenv.axon_hooks"] = mod
    try:
        from trn_agent_boot.trn_boot import _ntff_profile_via_ctypes
        mod.set_axon_ntff_profile_hook(
            _ntff_profile_via_ctypes("/opt/axon/libaxon_pjrt.so")
        )
    except Exception:
        pass


LAST_EXEC_NS = None


def kernel(x, wq, wk, wv, wo, q_norm_w, k_norm_w):
    global LAST_EXEC_NS
    from concourse import bass_utils

    x = np.asarray(x)
    if "nc" not in _CACHED:
        _CACHED["nc"] = build_nc()
    nc = _CACHED["nc"]

    in_maps = _host_prep(
        np.asarray(x, np.float32), np.asarray(wq, np.float32),
        np.asarray(wk, np.float32), np.asarray(wv, np.float32),
        np.asarray(wo, np.float32), np.asarray(q_norm_w, np.float32),
        np.asarray(k_norm_w, np.float32),
    )
    trace = bool(int(os.environ.get("BASS_KERNEL_TRACE", "0")))
    if trace:
        _install_ntff_shim()
    res = bass_utils.run_bass_kernel_spmd(
        nc, in_maps, core_ids=list(range(8)), trace=trace
    )
    LAST_EXEC_NS = res.exec_time_ns
    y = np.zeros((B, S, D), np.float32)
    for c in range(8):
        y[c // 4] += res.results[c]["y"]
    return y


# revision 25
# speedup vs baseline: 1.1338x; 1.0064x over previous
"""Trainium2 Bass kernel for GQA attention block (nn_Attention_81372450390110).

Module: y = AttnOut(x) with q/k RMSNorm + interleaved RoPE + causal GQA
(NH=16 q heads, KVH=4 kv heads, HD=128, D=2048, B=2, S=2048).

Sharding: 8 cores = 2 batches x 4 KV groups. Core c handles batch c//4 and
KV group c%4 (4 q heads + 1 kv head). Each core computes a full [S, D]
partial of the output projection (row-parallel over heads); the host sums
the 4 group-partials per batch.

Layout strategy (all feature-major, "transposed"):
  - host passes xT = x[b].T so the D contraction dim lands on partitions
  - qT/kT computed as [HD, S] directly (lhsT = weight chunk)
  - scores computed transposed: sT[k, q] = kT_blk.T @ qT_blk
  - softmax without max-subtraction (rmsnorm bounds |scores| <= sqrt(HD))
  - P@V untransposed (lhsT = P chunk, rhs = v block augmented with a ones
    column) -> out [q, HD | l]: the softmax denominator l falls out as the
    129th column, normalized via per-partition tensor_scalar, then the
    [128,128] tile is PE-transposed into attn_outT for the o-projection
  - cross-partition sums/broadcasts via ones-matmuls on TensorE
  - RoPE+norm-weight folded into host-precomputed coefficient tiles, with
    an even/odd deinterleaving permutation baked into wq/wk columns
"""

import os
import sys

sys.path.insert(0, "/opt/trn_rl_repo")

import numpy as np
import ml_dtypes

BF16 = ml_dtypes.bfloat16

B = 2
S = 2048
D = 2048
NH = 16
KVH = 4
HD = 128
THETA = 10000.0
EPS = 1e-6
NHL = NH // KVH  # q heads per core (4)
SCALE = 1.0 / float(np.sqrt(HD))

_CACHED = {}


def build_nc(s=S, d=D, nhl=NHL, hd=HD):
    import concourse.mybir as mybir
    import concourse.tile as tile
    from concourse import bacc

    f32 = mybir.dt.float32
    bf16 = mybir.dt.bfloat16
    AF = mybir.ActivationFunctionType

    kc_n = d // 128          # contraction chunks for projections
    nb_n = s // 512          # 512-token blocks
    qt_n = s // 512          # q tiles (512 wide) in attention
    kb_n = s // 128          # k blocks (128 wide)

    nc = bacc.Bacc("TRN2", target_bir_lowering=False, debug=False)

    xT_d = nc.dram_tensor("xT", (d, s), bf16, kind="ExternalInput")
    wq_d = nc.dram_tensor("wq", (d, nhl * hd), bf16, kind="ExternalInput")
    wk_d = nc.dram_tensor("wk", (d, hd), bf16, kind="ExternalInput")
    wv_d = nc.dram_tensor("wv", (d, hd), bf16, kind="ExternalInput")
    wo_d = nc.dram_tensor("wo", (nhl * hd, d), bf16, kind="ExternalInput")
    m1q_d = nc.dram_tensor("m1q", (hd, s), bf16, kind="ExternalInput")
    m2q_d = nc.dram_tensor("m2q", (hd, s), bf16, kind="ExternalInput")
    m1k_d = nc.dram_tensor("m1k", (hd, s), bf16, kind="ExternalInput")
    m2k_d = nc.dram_tensor("m2k", (hd, s), bf16, kind="ExternalInput")
    tri_d = nc.dram_tensor("tri", (128, 128), bf16, kind="ExternalInput")
    y_d = nc.dram_tensor("y", (s, d), f32, kind="ExternalOutput")

    with tile.TileContext(nc) as tc, nc.allow_low_precision(
        reason="bf16 compute by design; fp32 accumulation in PSUM"
    ):
        with (
            tc.tile_pool(name="const", bufs=1) as const,
            tc.tile_pool(name="persist", bufs=1) as persist,
        ):
            # ---- resident weights / coefficients -------------------------
            wq_sb = persist.tile([128, kc_n, nhl * hd], bf16, tag="wq")
            wq_re = wq_d.rearrange("(kc p) m -> p kc m", p=128)
            for kc in range(kc_n):
                nc.sync.dma_start(wq_sb[:, kc, :], wq_re[:, kc, :])
            wk_sb = persist.tile([128, kc_n, hd], bf16, tag="wk")
            wk_re = wk_d.rearrange("(kc p) m -> p kc m", p=128)
            wv_sb = persist.tile([128, kc_n, hd], bf16, tag="wv")
            wv_re = wv_d.rearrange("(kc p) m -> p kc m", p=128)
            for kc in range(kc_n):
                nc.sync.dma_start(wk_sb[:, kc, :], wk_re[:, kc, :])
                nc.sync.dma_start(wv_sb[:, kc, :], wv_re[:, kc, :])
            wo_sb = persist.tile([128, nhl, d], bf16, tag="wo")
            wo_re = wo_d.rearrange("(h p) m -> p h m", p=128)

            m1q_sb = persist.tile([hd, s], bf16, tag="m1q")
            m2q_sb = persist.tile([hd, s], bf16, tag="m2q")
            m1k_sb = persist.tile([hd, s], bf16, tag="m1k")
            m2k_sb = persist.tile([hd, s], bf16, tag="m2k")
            tri_sb = const.tile([128, 128], bf16, tag="tri")
            nc.sync.dma_start(tri_sb[:], tri_d[:])

            ones_k = const.tile([128, 1], bf16, tag="ones_k")
            nc.vector.memset(ones_k[:], 1.0)
            ones_1 = const.tile([1, 128], bf16, tag="ones_1")
            nc.vector.memset(ones_1[:], 1.0)
            eps_sb = const.tile([128, 1], f32, tag="eps")
            nc.vector.memset(eps_sb[:], EPS)
            ident = const.tile([128, 128], bf16, tag="ident")
            from concourse.masks import make_identity
            make_identity(nc, ident[:])

            # ---- persistent activations ---------------------------------
            qT_sb = [persist.tile([hd, s], bf16, tag=f"qT{h}", name=f"qT{h}") for h in range(nhl)]
            kT_sb = persist.tile([hd, s], bf16, tag="kT")
            v_sb = persist.tile([128, kb_n, hd + 1], bf16, tag="v")
            nc.vector.memset(v_sb[:, :, hd:hd + 1], 1.0)
            attT_sb = [persist.tile([hd, s], bf16, tag=f"attT{h}", name=f"attT{h}") for h in range(nhl)]

            # ================= Phase A: projections + norm + rope =========
            with (
                tc.tile_pool(name="xtp", bufs=2) as xtp,
                tc.tile_pool(name="workA", bufs=3) as wa,
                tc.tile_pool(name="psA", bufs=2, space="PSUM") as psA,
            ):
                xT_re = xT_d.rearrange("(kc p) n -> p kc n", p=128)

                def norm_rope_chain(q_ps, t, cs):
                    # rmsnorm via ones-matmul + bcast-matmul, rope via coeff
                    # tiles; PE ops here are emitted one tensor late so the
                    # ACT/DVE chain overlaps the next tensor's projection MMs.
                    sq = wa.tile([128, 512], bf16, tag="sq", name="sq")
                    nc.scalar.activation(sq[:], q_ps[:], AF.Square)
                    ssq = psA.tile([1, 512], f32, tag="ssq", name="ssq", bufs=1)
                    nc.tensor.matmul(ssq[:], ones_k[:], sq[:])
                    ssq_bf = wa.tile([1, 512], bf16, tag="ssq_bf", name="ssq_bf")
                    nc.scalar.copy(ssq_bf[:], ssq[:])
                    rb_ps = psA.tile([128, 512], f32, tag="rb_ps", name="rb_ps", bufs=1)
                    nc.tensor.matmul(rb_ps[:], ones_1[:], ssq_bf[:])
                    tmp = wa.tile([128, 512], f32, tag="tmp", name="tmp")
                    nc.scalar.activation(
                        tmp[:], rb_ps[:], AF.Sqrt, scale=1.0 / hd, bias=eps_sb[:]
                    )
                    rb = wa.tile([128, 512], f32, tag="rb", name="rb")
                    nc.vector.reciprocal_approx_fast(rb[:], tmp[:])
                    qn = wa.tile([128, 512], f32, tag="qn", name="qn")
                    nc.vector.tensor_mul(qn[:], q_ps[:], rb[:])
                    qs = wa.tile([128, 512], f32, tag="qs", name="qs")
                    nc.sync.dma_start(qs[0:64, :], qn[64:128, :])
                    nc.sync.dma_start(qs[64:128, :], qn[0:64, :])
                    m1 = m1q_sb if t < nhl else m1k_sb
                    m2 = m2q_sb if t < nhl else m2k_sb
                    t1 = wa.tile([128, 512], f32, tag="t1", name="t1")
                    nc.vector.tensor_mul(t1[:], qn[:], m1[:, cs])
                    t2 = wa.tile([128, 512], f32, tag="t2", name="t2")
                    nc.vector.tensor_mul(t2[:], qs[:], m2[:, cs])
                    dest = qT_sb[t] if t < nhl else kT_sb
                    nc.vector.tensor_add(dest[:, cs], t1[:], t2[:])

                pending = None
                for nb in range(nb_n):
                    cs = slice(nb * 512, (nb + 1) * 512)
                    xt = xtp.tile([128, kc_n, 512], bf16, tag="xt")
                    for kc in range(kc_n):
                        nc.sync.dma_start(xt[:, kc, :], xT_re[:, kc, cs])
                    if nb == 0:
                        nc.sync.dma_start(m1q_sb[:], m1q_d[:])
                        nc.sync.dma_start(m2q_sb[:], m2q_d[:])
                        nc.sync.dma_start(m1k_sb[:], m1k_d[:])
                        nc.sync.dma_start(m2k_sb[:], m2k_d[:])

                    # q heads then k: projection MMs now, norm chain deferred
                    for t in list(range(nhl)) + [nhl]:
                        q_ps = psA.tile([128, 512], f32, tag="q_ps", bufs=3)
                        for kc in range(kc_n):
                            if t < nhl:
                                lhsT = wq_sb[:, kc, t * hd:(t + 1) * hd]
                            else:
                                lhsT = wk_sb[:, kc, :]
                            nc.tensor.matmul(
                                q_ps[:], lhsT, xt[:, kc, :],
                                start=(kc == 0), stop=(kc == kc_n - 1),
                            )
                        if pending is not None:
                            norm_rope_chain(*pending)
                        pending = (q_ps, t, cs)

                    if nb == nb_n - 1 and pending is not None:
                        norm_rope_chain(*pending)
                        pending = None
                    # v: plain projection, token-major
                    for tt in range(4):
                        v_ps = psA.tile([128, hd], f32, tag="v_ps", bufs=2)
                        for kc in range(kc_n):
                            nc.tensor.matmul(
                                v_ps[:],
                                xt[:, kc, tt * 128:(tt + 1) * 128],
                                wv_sb[:, kc, :],
                                start=(kc == 0), stop=(kc == kc_n - 1),
                            )
                        nc.vector.tensor_copy(v_sb[:, nb * 4 + tt, 0:hd], v_ps[:])
                if pending is not None:
                    norm_rope_chain(*pending)

            # ================= Phase B: causal flash attention ============
            with (
                tc.tile_pool(name="workB", bufs=3) as wb,
                tc.tile_pool(name="psB", bufs=2, space="PSUM") as psB,
            ):
                for h in range(nhl):
                    nc.sync.dma_start(wo_sb[:, h, :], wo_re[:, h, :])
                for qt in range(qt_n):
                    for h in range(nhl):
                        qcs = slice(qt * 512, (qt + 1) * 512)
                        nkb = 4 * qt + 4
                        att_ps = [
                            psB.tile([128, hd + 1], f32, tag="att", bufs=4,
                                     name=f"att{qs}")
                            for qs in range(4)
                        ]
                        s_tiles = {}

                        def emit_s(kb):
                            sp = psB.tile([128, 512], f32, tag="s_ps", name="s_ps", bufs=3)
                            r = kb - 4 * qt
                            c0 = 128 * r if r > 0 else 0
                            nc.tensor.matmul(
                                sp[:, c0:512],
                                kT_sb[:, kb * 128:(kb + 1) * 128],
                                qT_sb[h][:, qt * 512 + c0:(qt + 1) * 512],
                            )
                            s_tiles[kb] = sp

                        emit_s(0)
                        if nkb > 1:
                            emit_s(1)
                        for kb in range(nkb):
                            if kb + 2 < nkb:
                                emit_s(kb + 2)
                            sp = s_tiles.pop(kb)
                            p = wb.tile([128, 512], bf16, tag="p", bufs=4)
                            r = kb - 4 * qt
                            if r >= 0:
                                nc.scalar.activation(
                                    p[:, 128 * r:512], sp[:, 128 * r:512],
                                    AF.Exp, scale=SCALE,
                                )
                                nc.vector.tensor_mul(
                                    p[:, 128 * r:128 * (r + 1)],
                                    p[:, 128 * r:128 * (r + 1)],
                                    tri_sb[:],
                                )
                            else:
                                nc.scalar.activation(p[:], sp[:], AF.Exp, scale=SCALE)
                            for qs in range(4):
                                kmax = 4 * qt + qs
                                if kb > kmax:
                                    continue
                                nc.tensor.matmul(
                                    att_ps[qs][:],
                                    p[:, qs * 128:(qs + 1) * 128],
                                    v_sb[:, kb, :],
                                    start=(kb == 0), stop=(kb == kmax),
                                )
                        for qs in range(4):
                            rec = wb.tile([128, 1], f32, tag="rec")
                            nc.vector.reciprocal(rec[:], att_ps[qs][:, hd:hd + 1])
                            att_n = wb.tile([128, 128], bf16, tag="att_n")
                            nc.vector.tensor_scalar_mul(
                                att_n[:], att_ps[qs][:, 0:hd], rec[:]
                            )
                            tp_ps = psB.tile([128, 128], bf16, tag="tp", bufs=1)
                            nc.tensor.transpose(tp_ps[:], att_n[:], ident[:])
                            nc.vector.tensor_copy(
                                attT_sb[h][:, qt * 512 + qs * 128:
                                           qt * 512 + (qs + 1) * 128],
                                tp_ps[:],
                            )
                        if h == nhl - 1:
                            for tt in range(qt * 4, qt * 4 + 4):
                                for db in range(d // 512):
                                    y_ps = psB.tile(
                                        [128, 512], f32, tag="s_ps", name="y_ps",
                                        bufs=3,
                                    )
                                    for hh in range(nhl):
                                        nc.tensor.matmul(
                                            y_ps[:],
                                            attT_sb[hh][:, tt * 128:(tt + 1) * 128],
                                            wo_sb[:, hh, db * 512:(db + 1) * 512],
                                            start=(hh == 0), stop=(hh == nhl - 1),
                                        )
                                    y_sb = wb.tile([128, 512], f32, tag="ysb",
                                                   name="ysb")
                                    nc.scalar.copy(y_sb[:], y_ps[:])
                                    nc.sync.dma_start(
                                        y_d[tt * 128:(tt + 1) * 128,
                                            db * 512:(db + 1) * 512],
                                        y_sb[:],
                                    )



    nc.compile()
    return nc


def _rope_coeffs(norm_w, s=S, hd=HD):
    """Coefficient tiles [hd, s] folding rope cos/sin + permuted norm weight."""
    perm = np.concatenate([np.arange(0, hd, 2), np.arange(1, hd, 2)])
    w = np.asarray(norm_w, np.float64)[perm]
    half = hd // 2
    pos = np.arange(s, dtype=np.float64)
    inv_freq = 1.0 / (THETA ** (np.arange(0, hd, 2, dtype=np.float64) / hd))
    ang = pos[None, :] * inv_freq[:, None]          # [half, s]
    cos, sin = np.cos(ang), np.sin(ang)
    m1 = np.empty((hd, s), np.float32)
    m2 = np.empty((hd, s), np.float32)
    m1[:half] = cos * w[:half, None]
    m1[half:] = cos * w[half:, None]
    m2[:half] = -sin * w[half:, None]
    m2[half:] = sin * w[:half, None]
    return m1, m2


def _host_prep(x, wq, wk, wv, wo, q_norm_w, k_norm_w):
    perm = np.concatenate([np.arange(0, HD, 2), np.arange(1, HD, 2)])
    m1q, m2q = _rope_coeffs(q_norm_w)
    m1k, m2k = _rope_coeffs(k_norm_w)
    tri = np.triu(np.ones((128, 128), np.float32)).astype(BF16)

    in_maps = []
    for c in range(8):
        b, g = c // 4, c % 4
        heads = range(NHL * g, NHL * g + NHL)
        wq_loc = np.concatenate(
            [wq[:, h * HD:(h + 1) * HD][:, perm] for h in heads], axis=1
        )
        in_maps.append({
            "xT": np.ascontiguousarray(x[b].T).astype(BF16),
            "wq": np.ascontiguousarray(wq_loc).astype(BF16),
            "wk": np.ascontiguousarray(wk[:, g * HD:(g + 1) * HD][:, perm]).astype(BF16),
            "wv": np.ascontiguousarray(wv[:, g * HD:(g + 1) * HD]).astype(BF16),
            "wo": np.ascontiguousarray(wo[NHL * g * HD:NHL * (g + 1) * HD, :]).astype(BF16),
            "m1q": m1q.astype(BF16), "m2q": m2q.astype(BF16),
            "m1k": m1k.astype(BF16), "m2k": m2k.astype(BF16),
            "tri": tri,
        })
    return in_maps


def _install_ntff_shim():
    import types
    if "antenv.axon_hooks" in sys.modules:
        return
    mod = types.ModuleType("antenv.axon_hooks")
    _hook = [None]
    mod.set_axon_ntff_profile_hook = lambda h: _hook.__setitem__(0, h)
    mod.get_axon_ntff_profile_hook = lambda: _hook[0]
    sys.modules["antenv.axon_hooks"] = mod
    try:
        from trn_agent_boot.trn_boot import _ntff_profile_via_ctypes
        mod.set_axon_ntff_profile_hook(
            _ntff_profile_via_ctypes("/opt/axon/libaxon_pjrt.so")
        )
    except Exception:
        pass


LAST_EXEC_NS = None


def kernel(x, wq, wk, wv, wo, q_norm_w, k_norm_w):
    global LAST_EXEC_NS
    from concourse import bass_utils

    x = np.asarray(x)
    if "nc" not in _CACHED:
        _CACHED["nc"] = build_nc()
    nc = _CACHED["nc"]

    in_maps = _host_prep(
        np.asarray(x, np.float32), np.asarray(wq, np.float32),
        np.asarray(wk, np.float32), np.asarray(wv, np.float32),
        np.asarray(wo, np.float32), np.asarray(q_norm_w, np.float32),
        np.asarray(k_norm_w, np.float32),
    )
    trace = bool(int(os.environ.get("BASS_KERNEL_TRACE", "0")))
    if trace:
        _install_ntff_shim()
    res = bass_utils.run_bass_kernel_spmd(
        nc, in_maps, core_ids=list(range(8)), trace=trace
    )
    LAST_EXEC_NS = res.exec_time_ns
    y = np.zeros((B, S, D), np.float32)
    for c in range(8):
        y[c // 4] += res.results[c]["y"]
    return y
